# revision 31
# baseline (speedup 1.0000x reference)
"""BiLSTM-CRF NLL kernel for 8 Trainium2 NeuronCores.

Strategy (3 SPMD launches, host glue between them):
  L1 "layer0": 8 cores = 2 dirs x 4 batch-quarters (16 seqs/core, one LSTM dir).
     Per core: gx = W_ih @ x^T (+biases) as chunked matmuls interleaved with the
     256-step recurrent scan (weight-stationary matmuls; gates in a
     [128-partition, batch-free] layout so elementwise ops use all lanes).
     The per-step serial chain is minimized with two runtime-registered custom
     DVE ops: ct = clamp(w+m) and h = sig_o*tanh_poly(2*ct), which removes the
     tanh ACTIVATE, two tensor_tensor ops, and their cross-engine syncs from
     the recurrence-critical path.
  L2 "layer1": same program shape with K=512 input; host reshards and handles
     the per-sequence reversal of the backward direction.
  L3 "logits+CRF": 8 cores = 8 batch-eighths. The 255-step serial CRF scan is
     replaced by 16 overlapped 36-phase window scans (4 partition-blocks x 4
     col-groups, block-diagonal exp(trans) matmuls): products of the positive
     transition operator contract to rank-1 in ~20 steps (Birkhoff), so each
     window recovers the true alpha direction after its 20-step overlap, and
     per-segment log-scales chain through left-vector (gamma window scan)
     inner products. Extraction at t=len-1, renorm ledgers and the scale chain
     are folded into host-built masks. start/end/transition/bias numerator
     terms are summed on host.

Matmuls run in bf16 (fp32 PSUM accumulate); LSTM cell state is fp32.
"""

import os
import sys

import numpy as np

for _p in ("/opt/trn_rl_repo", "/root/.axon_site/_ro/trn_rl_repo"):
    if _p not in sys.path and os.path.isdir(_p):
        sys.path.insert(0, _p)

import ml_dtypes  # noqa: E402

BF16 = ml_dtypes.bfloat16

B, T, V, E, HD, NT = 64, 256, 50000, 256, 256, 20
NCORES = 8
BL = 16            # sequences per core in L1/L2 (batch quarter)
BC = 8             # sequences per core in L3 (batch eighth)
NTOK = BL * T      # tokens per core in L1/L2
NTOK3 = BC * T     # tokens per core in L3
NJ = 8             # gate tiles of 128 rows (4 gates x 256 HD / 128)
NCH = 512          # matmul N-chunk (tokens)
TCH = NCH // BL    # timesteps per gx chunk (32)
RENORM_EVERY = 8   # CRF renormalization interval
NREN = (T - 1) // RENORM_EVERY   # renorm slots used (t = 8,16,...,248)

# gate order stays pytorch-native (i,f,g,o): the c-path gates (i,f,g) are
# tiles 0..5 (one contiguous sigmoid), o is tiles 6..7 (deferred off the
# critical path). g rows are pre-scaled by 2 so tanh(x) = 2*sig(2x)-1.
_PERM = np.arange(4 * HD)

# cell state kept as ct = c/2 so ct = sig_f*ct + w (w = sig_i*(sig(2g)-0.5));
# tanh(c) = tanh(2*ct) evaluated as a clamped odd quintic on the DVE, which
# removes the tanh ACTIVATE (and a cross-engine sync) from the serial chain.
CLAMP_B = 1.25                    # ct clamp (real c clamped at 2*CLAMP_B)
# odd-quintic minimax fit of tanh(2x) on [0, CLAMP_B] (see _fit_tanh2)
TANH5_COEF = None                 # filled by _fit_tanh2() at import
USE_TH3 = True                    # fuse h = sig_o*tanh (deg-3) into one op

_CACHE = {}
LAST_RESULTS = []   # BassKernelResults of the launches of the last kernel() call


def _fit_tanh2(Bc):
    x = np.linspace(1e-6, Bc, 4001, dtype=np.float64)
    t = np.tanh(2.0 * x)
    q = x * x
    A = np.stack([x, x * q, x * q * q], 1)
    w = np.ones_like(x)
    sol = None
    for _ in range(80):
        sol, *_ = np.linalg.lstsq(A * w[:, None], t * w, rcond=None)
        err = A @ sol - t
        w = 1.0 + 10.0 * np.abs(err) / np.abs(err).max()
    return [float(v) for v in sol]


TANH5_COEF = _fit_tanh2(CLAMP_B)


def _fit_tanh2_deg3(Bc):
    x = np.linspace(1e-6, Bc, 4001, dtype=np.float64)
    t = np.tanh(2.0 * x)
    q = x * x
    A = np.stack([x, x * q], 1)
    w = np.ones_like(x)
    sol = None
    for _ in range(100):
        sol, *_ = np.linalg.lstsq(A * w[:, None], t * w, rcond=None)
        err = A @ sol - t
        w = 1.0 + 12.0 * np.abs(err) / np.abs(err).max()
    return [float(v) for v in sol]


TANH3_COEF = _fit_tanh2_deg3(CLAMP_B)

_DVE_OPS = {}


def _register_dve_ops():
    """Runtime-register the two fused cell ops with the custom-DVE table."""
    if _DVE_OPS:
        return _DVE_OPS
    from concourse import dve_ops as DVO
    from concourse.dve_spec import (
        Spec, Src0, Src1, C0, C1, C2, Zero, maxx, minn, lower)
    from concourse.dve_spec import _has_src1 as has_src1
    from concourse.dve_uop import DveOpSpec

    def make(name, spec):
        if name in DVO.CUSTOM_DVE_SPECS:
            return next(op for op in DVO.OPS if op.name == name)
        row = DVO._CUSTOM_DVE_ROW_BASE + len(DVO.OPS)
        DVO._SUB_OPCODE_FOR_NAME[name] = row
        shas = {}
        for ver in ("v3", "v4"):
            uops = lower(spec, ver=ver)
            shas[ver] = DveOpSpec(name=name, opcode=row, uops=uops,
                                  rd1_en=has_src1(spec)).sha(ver)
        op = DVO.DveOp(name, spec, subdim=False, uops_sha=shas)
        DVO.OPS.append(op)
        DVO.CUSTOM_DVE_SPECS[name] = spec
        return op

    # ct = clamp(w + m, -C0, C0)
    s = Src0 + Src1
    body_ca = minn(maxx(s, Zero - C0), C0)
    clampadd = make("ANT_CLAMPADD", Spec(
        body=body_ca,
        reference=lambda in0, in1, s0: np.clip(
            in0.astype(np.float32) + in1.astype(np.float32), -s0, s0),
    ))

    # out = x*(C0 + C1*x^2 + C2*x^4)   (odd quintic; input pre-clamped)
    q = Src0 * Src0
    body_t5 = Src0 * ((C2 * q + C1) * q + C0)
    tanh5 = make("ANT_TANH5", Spec(
        body=body_t5,
        reference=lambda in0, s0, s1, imm2: (
            in0 * (s0 + s1 * in0 * in0 + imm2 * (in0 * in0) ** 2)
        ).astype(np.float32),
    ))
    # h = sig_o * tanh3(ct): odd cubic with the gate product fused
    q3 = Src0 * Src0
    body_t3 = Src1 * (Src0 * (C1 * q3 + C0))
    th3mul = make("ANT_TH3MUL", Spec(
        body=body_t3,
        reference=lambda in0, in1, s0, s1: (
            in1 * in0 * (s0 + s1 * in0 * in0)).astype(np.float32),
    ))
    _DVE_OPS["clampadd"] = clampadd
    _DVE_OPS["tanh5"] = tanh5
    _DVE_OPS["th3mul"] = th3mul
    return _DVE_OPS


def _mods():
    import concourse.bass as bass
    import concourse.tile as tile
    from concourse import bacc, mybir
    from concourse.bass_utils import run_bass_kernel_spmd
    return bass, tile, bacc, mybir, run_bass_kernel_spmd


def _install_ntff_shim():
    """Provide antenv.axon_hooks (missing in this image) so that
    run_bass_kernel_spmd(trace=True) can capture NTFF profiles through
    libaxon_pjrt.so. Mirrors trn_agent_boot._ntff_profile_via_ctypes."""
    import sys as _sys
    if "antenv.axon_hooks" in _sys.modules:
        return
    import contextlib
    import ctypes
    import types

    so_path = "/opt/axon/libaxon_pjrt.so"
    mod = types.ModuleType("antenv.axon_hooks")
    _hook_box = [None]

    def set_axon_ntff_profile_hook(h):
        _hook_box[0] = h

    def get_axon_ntff_profile_hook():
        return _hook_box[0]

    mod.set_axon_ntff_profile_hook = set_axon_ntff_profile_hook
    mod.get_axon_ntff_profile_hook = get_axon_ntff_profile_hook
    _sys.modules["antenv.axon_hooks"] = mod

    try:
        lib = ctypes.CDLL(so_path)
        if not hasattr(lib, "axon_start_nrt_profile"):
            return
        lib.axon_start_nrt_profile.argtypes = [
            ctypes.POINTER(ctypes.c_int64), ctypes.c_size_t]
        lib.axon_start_nrt_profile.restype = ctypes.c_int64
        lib.axon_stop_nrt_profile.argtypes = [ctypes.c_char_p]
        lib.axon_stop_nrt_profile.restype = ctypes.c_int64

        @contextlib.contextmanager
        def _hook(output_dir, device_ids):
            import jax
            jax.devices()
            if device_ids:
                ids = (ctypes.c_int64 * len(device_ids))(*device_ids)
                rc = lib.axon_start_nrt_profile(ids, len(device_ids))
            else:
                rc = lib.axon_start_nrt_profile(None, 0)
            if rc != 0:
                raise RuntimeError(f"axon_start_nrt_profile rc={rc}")
            try:
                yield
            finally:
                n = lib.axon_stop_nrt_profile(str(output_dir).encode())
                print(f"profile: {n} file(s) written to {output_dir}",
                      file=sys.stderr)

        set_axon_ntff_profile_hook(_hook)
    except OSError:
        pass


# --------------------------------------------------------------------------
# program builders
# --------------------------------------------------------------------------

def build_layer_program(kc_in):
    """One BiLSTM direction for BL sequences. kc_in = input dim / 128."""
    bass, tile, bacc, mybir, _ = _mods()
    dt = mybir.dt
    AF = mybir.ActivationFunctionType
    AO = mybir.AluOpType
    dve = _register_dve_ops()
    a0, a1, a2 = TANH5_COEF

    nc = bacc.Bacc("TRN2", target_bir_lowering=False, debug=False)
    xT = nc.dram_tensor("xT", [kc_in, 128, NTOK], dt.bfloat16, kind="ExternalInput").ap()
    wih = nc.dram_tensor("wih", [kc_in, 128, 4 * HD], dt.bfloat16, kind="ExternalInput").ap()
    whh = nc.dram_tensor("whh", [2, 128, 4 * HD], dt.bfloat16, kind="ExternalInput").ap()
    bias = nc.dram_tensor("bias", [128, NJ], dt.float32, kind="ExternalInput").ap()
    hout = nc.dram_tensor("hout", [128, 2, T, BL], dt.bfloat16,
                          kind="ExternalOutput").ap()

    NCHUNKS = NTOK // NCH  # 8

    with tile.TileContext(nc) as tc:
        with (
            tc.tile_pool(name="w", bufs=1) as wpool,
            tc.tile_pool(name="big", bufs=1) as big,
            tc.tile_pool(name="gxp", bufs=2) as gxp,
            tc.tile_pool(name="xs", bufs=3) as xs,
            tc.tile_pool(name="st", bufs=1) as st,
            tc.tile_pool(name="ew", bufs=4) as ew,
            tc.tile_pool(name="ps", bufs=2, space="PSUM") as ps,
            tc.tile_pool(name="psg", bufs=3, space="PSUM") as psg,
        ):
            wih_sb = wpool.tile([128, kc_in, 4 * HD], dt.bfloat16)
            whh_sb = wpool.tile([128, 2, 4 * HD], dt.bfloat16)
            bias_sb = wpool.tile([128, NJ], dt.float32)
            # bias first (tiny, every gx block needs it), then wih j-slice by
            # j-slice so gx_block(0) starts before the full weight transfer
            # lands; whh is only needed once the first scan step runs
            nc.sync.dma_start(bias_sb[:], bias[:])
            for j in range(NJ):
                for kc in range(kc_in):
                    nc.sync.dma_start(
                        wih_sb[:, kc, j * 128:(j + 1) * 128],
                        wih[kc, :, j * 128:(j + 1) * 128])
            for kc in range(2):
                nc.sync.dma_start(whh_sb[:, kc, :], whh[kc])
            from concourse.masks import make_identity
            ident = wpool.tile([128, 128], dt.bfloat16)
            make_identity(nc, ident[:])

            hist = big.tile([128, 2, T + 1, BL], dt.bfloat16)
            cst = st.tile([128, 2, BL], dt.float32)
            nc.vector.memset(hist[:, :, 0, :], 0.0)
            nc.vector.memset(cst[:], 0.0)

            # gx compute for chunk n, interleaved into chunk n-1's scan steps
            def gx_block(gxb, xc, j):
                acc = psg.tile([128, NCH], dt.float32, name="acc")
                for kc in range(kc_in):
                    nc.tensor.matmul(
                        acc[:], wih_sb[:, kc, j * 128:(j + 1) * 128],
                        xc[:, kc, :],
                        start=(kc == 0), stop=(kc == kc_in - 1))
                accv = acc[:].rearrange("p (t b) -> p t b", b=BL)
                # scalar engine: with tanh moved to the DVE the scalar engine
                # has slack, and the DVE FIFO is now the per-step chain pole
                nc.scalar.add(gxb[:, j], accv, bias_sb[:, j:j + 1])

            def load_x(n):
                xc = xs.tile([128, kc_in, NCH], dt.bfloat16, name="xc")
                for kc in range(kc_in):
                    nc.sync.dma_start(xc[:, kc, :],
                                      xT[kc, :, n * NCH:(n + 1) * NCH])
                return xc

            def prefill(gxb, tt):
                # identity matmuls drop gx(+bias) for a whole step into PSUM;
                # o-gates go to their own bank so the c-path sigmoid is not
                # gated on them (PSUM deps are bank-granular)
                G1 = ps.tile([128, 6, BL], dt.float32, name="G1")
                nc.tensor.matmul(G1[:], ident[:], gxb[:, 0:6, tt, :],
                                 start=True, stop=False, skip_group_check=True)
                G2 = ps.tile([128, 2, BL], dt.float32, name="G2")
                nc.tensor.matmul(G2[:], ident[:], gxb[:, 6:8, tt, :],
                                 start=True, stop=False, skip_group_check=True)
                return G1, G2

            # chunk 0 gx up front
            xc_cur = load_x(0)
            gx_cur = gxp.tile([128, NJ, TCH, BL], dt.bfloat16, name="gxb")
            for j in range(NJ):
                gx_block(gx_cur, xc_cur, j)
            Gc = prefill(gx_cur, 0)

            # scan; cell (g rows pre-scaled by 2 on host):
            #   w = (sig_g' - 0.5) * sig_i ; c = 2w + sig_f*c ; h = sig_o*tanh(c)
            for n in range(NCHUNKS):
                gx_nxt = None
                if n + 1 < NCHUNKS:
                    xc_nxt = load_x(n + 1)
                    gx_nxt = gxp.tile([128, NJ, TCH, BL], dt.bfloat16,
                                      name="gxb")
                for tt in range(TCH):
                    t = n * TCH + tt
                    G1, G2 = Gc
                    # c-path gates (i,f,g) first; kc-major so the kc=0 MMs can
                    # start as soon as the kc=0 half of h(t-1) is written
                    for kc in range(2):
                        for j in range(6):
                            nc.tensor.matmul(
                                G1[:, j, :], whh_sb[:, kc, j * 128:(j + 1) * 128],
                                hist[:, kc, t, :], start=False,
                                stop=(j == 5 and kc == 1),
                                skip_group_check=True)
                    # o-gate matmuls + its sigmoid run off the critical path
                    for kc in range(2):
                        for j in (6, 7):
                            nc.tensor.matmul(
                                G2[:, j - 6, :],
                                whh_sb[:, kc, j * 128:(j + 1) * 128],
                                hist[:, kc, t, :], start=False,
                                stop=(j == 7 and kc == 1),
                                skip_group_check=True)
                    # prefill next step's PSUM + spread next chunk's gx matmuls
                    if tt + 1 < TCH:
                        Gc = prefill(gx_cur, tt + 1)
                    elif gx_nxt is not None:
                        Gc = prefill(gx_nxt, 0)
                    if gx_nxt is not None and tt % 4 == 1 and tt // 4 < NJ:
                        gx_block(gx_nxt, xc_nxt, tt // 4)

                    A1 = ew.tile([128, 6, BL], dt.float32, name="A1")
                    nc.scalar.activation(A1[:], G1[:], AF.Sigmoid)
                    A2 = ew.tile([128, 2, BL], dt.float32, name="A2")
                    nc.scalar.activation(A2[:], G2[:], AF.Sigmoid)
                    w = ew.tile([128, 2, BL], dt.float32, name="w")
                    nc.vector.scalar_tensor_tensor(
                        w[:], A1[:, 4:6, :], 0.5, A1[:, 0:2, :],
                        AO.subtract, AO.mult)
                    m1 = ew.tile([128, 2, BL], dt.float32, name="m1")
                    nc.vector.tensor_tensor(m1[:], A1[:, 2:4, :], cst[:],
                                            AO.mult)
                    # ct = clamp(w + m, +-B); tanh(2*ct) as odd polynomial and
                    # the sig_o product fused on the DVE — no tanh ACTIVATE or
                    # extra sync on the chain
                    nc.vector._custom_dve(dve["clampadd"], out=cst[:],
                                          in0=w[:], in1=m1[:], s0=CLAMP_B)
                    if USE_TH3:
                        b0, b1c = TANH3_COEF
                        # single fused h write: a kc-split variant (earlier
                        # kc=0 matmul start) measured 29us slower overall —
                        # the extra DVE op's fixed cost outweighs the overlap
                        nc.vector._custom_dve(
                            dve["th3mul"], out=hist[:, :, t + 1, :],
                            in0=cst[:], in1=A2[:], s0=b0, s1=b1c)
                    else:
                        Tc = ew.tile([128, 2, BL], dt.float32, name="Tc")
                        nc.vector._custom_dve(dve["tanh5"], out=Tc[:],
                                              in0=cst[:], s0=a0, s1=a1,
                                              imm2=a2)
                        nc.vector.tensor_tensor(hist[:, 0, t + 1, :],
                                                A2[:, 0, :], Tc[:, 0, :],
                                                AO.mult)
                        nc.vector.tensor_tensor(hist[:, 1, t + 1, :],
                                                A2[:, 1, :], Tc[:, 1, :],
                                                AO.mult)
                if gx_nxt is not None:
                    gx_cur, xc_cur = gx_nxt, xc_nxt
                # stream finished history out
                t0 = n * TCH
                nc.sync.dma_start(hout[:, :, t0:t0 + TCH, :],
                                  hist[:, :, t0 + 1:t0 + TCH + 1, :])
    nc.compile()
    return nc


# ---- CRF v2: rank-1 overlapped-segment scan -------------------------------
# 16 time-segments of 16 steps, laid out as 4 partition-blocks (base 0/32/64/
# 96) x 4 col-groups. Each segment scans a 36-phase window (20-step overlap
# into its predecessor) from a uniform init; Birkhoff contraction makes the
# windowed transfer operator rank-1 (~0.42/step), so the true alpha direction
# is recovered after the overlap and per-segment log-scale factors chain via
# left-vector inner products (gamma scans over the overlap windows).
S2, LSEG, W2, LOV = 16, 16, 36, 20
NSL = 4                      # renorm ledger slots (after phases 7/15/23/31)
WST = [0 if s < 2 else 16 * s - LOV for s in range(S2)]
SRC_SLOT = {s: ((0, 11) if s == 2 else (1, 27) if s == 3 else (s - 2, 31))
            for s in range(2, S2)}


def build_crf_program_v2():
    bass, tile, bacc, mybir, _ = _mods()
    dt = mybir.dt
    AF = mybir.ActivationFunctionType
    AO = mybir.AluOpType

    nc = bacc.Bacc("TRN2", target_bir_lowering=False, debug=False)
    hcat = nc.dram_tensor("hcat", [4, 128, NTOK3], dt.bfloat16, kind="ExternalInput").ap()
    linw = nc.dram_tensor("linw", [4, 128, NT], dt.bfloat16, kind="ExternalInput").ap()
    linb4 = nc.dram_tensor("linb4", [128, 1], dt.float32, kind="ExternalInput").ap()
    e4 = nc.dram_tensor("e4", [128, 128], dt.bfloat16, kind="ExternalInput").ap()
    e4t = nc.dram_tensor("e4t", [128, 128], dt.bfloat16, kind="ExternalInput").ap()
    v0 = nc.dram_tensor("v0", [128, 4, BC], dt.bfloat16, kind="ExternalInput").ap()
    g0 = nc.dram_tensor("g0", [128, 4, BC], dt.bfloat16, kind="ExternalInput").ap()
    b1 = nc.dram_tensor("b1", [128, 4], dt.bfloat16, kind="ExternalInput").ap()
    b1t = nc.dram_tensor("b1t", [4, 128], dt.bfloat16, kind="ExternalInput").ap()
    ident = nc.dram_tensor("identb", [128, 128], dt.bfloat16, kind="ExternalInput").ap()
    selz = nc.dram_tensor("selz", [128, BC, W2, 4], dt.bfloat16, kind="ExternalInput").ap()
    emmask = nc.dram_tensor("emmask", [128, 4, W2 * BC], dt.bfloat16, kind="ExternalInput").ap()
    mchain = nc.dram_tensor("mchain", [4, BC, 4], dt.float32, kind="ExternalInput").ap()
    mled = nc.dram_tensor("mled", [4, BC, NSL, 4], dt.float32, kind="ExternalInput").ap()
    ones4 = nc.dram_tensor("ones4", [4, 1], dt.float32, kind="ExternalInput").ap()
    ones128 = nc.dram_tensor("ones128", [128, 1], dt.float32, kind="ExternalInput").ap()
    part_out = nc.dram_tensor("part_out", [1, BC], dt.float32, kind="ExternalOutput").ap()
    emit_out = nc.dram_tensor("emit_out", [1, 1], dt.float32, kind="ExternalOutput").ap()

    WB = W2 * BC  # 288 cols per (pb, cg) logits slab

    with tile.TileContext(nc) as tc:
        with (
            tc.tile_pool(name="w", bufs=1) as wp,
            tc.tile_pool(name="big", bufs=1) as big,
            tc.tile_pool(name="sm", bufs=4) as sm,
            tc.tile_pool(name="pslg", bufs=2, space="PSUM") as pslg,
            tc.tile_pool(name="psv", bufs=2, space="PSUM") as psv,
            tc.tile_pool(name="psg", bufs=2, space="PSUM") as psg,
            tc.tile_pool(name="psx", bufs=1, space="PSUM") as psx,
        ):
            # two shared PSUM scratch tiles (bank-quantized: 8 banks total)
            aux = psx.tile([128, 4 * BC], dt.float32, name="aux")
            aux2 = psx.tile([128, 4 * BC], dt.float32, name="aux2")
            hc_sb = big.tile([128, 4, NTOK3], dt.bfloat16)
            for kc in range(4):
                nc.sync.dma_start(hc_sb[:, kc, :], hcat[kc])
            lw_sb = wp.tile([128, 4, NT], dt.bfloat16)
            for kc in range(4):
                nc.sync.dma_start(lw_sb[:, kc, :], linw[kc])
            lb_sb = wp.tile([128, 1], dt.float32)
            nc.sync.dma_start(lb_sb[:], linb4[:])
            e4_sb = wp.tile([128, 128], dt.bfloat16)
            nc.sync.dma_start(e4_sb[:], e4[:])
            e4t_sb = wp.tile([128, 128], dt.bfloat16)
            nc.sync.dma_start(e4t_sb[:], e4t[:])
            v0_sb = wp.tile([128, 4, BC], dt.bfloat16)
            nc.sync.dma_start(v0_sb[:], v0[:])
            g0_sb = wp.tile([128, 4, BC], dt.bfloat16)
            nc.sync.dma_start(g0_sb[:], g0[:])
            b1_sb = wp.tile([128, 4], dt.bfloat16)
            nc.sync.dma_start(b1_sb[:], b1[:])
            b1t_sb = wp.tile([4, 128], dt.bfloat16)
            nc.sync.dma_start(b1t_sb[:], b1t[:])
            id_sb = wp.tile([128, 128], dt.bfloat16)
            nc.sync.dma_start(id_sb[:], ident[:])
            selz_sb = big.tile([128, BC, W2, 4], dt.bfloat16)
            nc.sync.dma_start(selz_sb[:], selz[:])
            em_sb = big.tile([128, 4, WB], dt.bfloat16)
            nc.sync.dma_start(em_sb[:], emmask[:])
            mch_sb = wp.tile([4, BC, 4], dt.float32)
            nc.sync.dma_start(mch_sb[:], mchain[:])
            mld_sb = wp.tile([4, BC, NSL, 4], dt.float32)
            nc.sync.dma_start(mld_sb[:], mled[:])
            o4_sb = wp.tile([4, 1], dt.float32)
            nc.sync.dma_start(o4_sb[:], ones4[:])
            o128_sb = wp.tile([128, 1], dt.float32)
            nc.sync.dma_start(o128_sb[:], ones128[:])

            # ---- logits: per-(pb,cg) PSUM slab -> scalar-engine copy to SBUF
            logits = big.tile([128, 4, WB], dt.float32)
            nc.vector.memset(logits[:], 0.0)
            for pb in range(4):
                for cg in range(4):
                    s = pb * 4 + cg
                    off = WST[s]
                    lgc = pslg.tile([128, 512], dt.float32, name="lgc")
                    outv = lgc[32 * pb:32 * pb + 20, 0:WB]
                    for kc in range(4):
                        rhs = hc_sb[:, kc, :].rearrange(
                            "p (t b) -> p t b", b=BC)[:, off:off + W2, :]
                        nc.tensor.matmul(
                            outv, lw_sb[:, kc, :], rhs,
                            start=(kc == 0), stop=(kc == 3),
                            tile_position=(0, 32 * pb),
                            skip_group_check=True)
                    nc.scalar.copy(
                        logits[32 * pb:32 * pb + 20, cg, :], outv)

            # exp(logits + bias) -> elog [128, 4, W*BC]
            elog = big.tile([128, 4, WB], dt.float32)
            nc.scalar.activation(elog[:], logits[:], AF.Exp, bias=lb_sb[:])

            # emission: sum over (logits .* emmask); dump product into hc_sb
            erow = sm.tile([128, 1], dt.float32)
            dump = hc_sb[:, 0, 0:4 * WB].rearrange("p (c w) -> p c w", c=4)
            nc.vector.scalar_tensor_tensor(
                dump, logits[:], 1.0, em_sb[:], AO.mult, AO.mult,
                accum_out=erow[:])
            etot = aux2[0:1, 0:1]
            nc.tensor.matmul(etot, o128_sb[:], erow[:], start=True, stop=True,
                             skip_group_check=True)
            eout = sm.tile([1, 1], dt.float32)
            nc.vector.tensor_copy(eout[:], etot)
            nc.sync.dma_start(emit_out[:], eout[:])

            elogv = elog[:].rearrange("p c (w b) -> p c w b", b=BC)

            # ---- forward + left window scans, interleaved emission so the
            # two independent chains overlap on the in-order engine FIFOs ----
            vhist = big.tile([128, W2, 4, BC], dt.bfloat16)
            shist = sm.tile([4, NSL, 4, BC], dt.float32)
            gcur = g0_sb          # SBUF bf16 gamma state (or PSUM view)
            gcur_psum = None
            gbuf = sm.tile([128, 4, BC], dt.bfloat16, name="gbuf")

            def left_phase(p, ip):
                nonlocal gcur, gcur_psum
                u = sm.tile([128, 4, BC], dt.bfloat16, name=f"gu{ip % 2}")
                src = gcur_psum if gcur_psum is not None else gcur[:]
                nc.vector.tensor_tensor(u[:], src, elogv[:, :, p, :], AO.mult)
                gy = psg.tile([128, 4 * BC], dt.float32, name="gy")
                nc.tensor.matmul(gy[:], e4t_sb[:],
                                 u[:].rearrange("p c b -> p (c b)"),
                                 start=True, stop=True)
                gcur_psum = gy[:].rearrange("p (c b) -> p c b", b=BC)
                if p % 8 == 0:
                    gsb = sm.tile([128, 4, BC], dt.bfloat16,
                                  name=f"gc{ip % 2}")
                    nc.vector.tensor_copy(gsb[:], gcur_psum)
                    ss2 = aux[0:4, :]
                    nc.tensor.matmul(ss2, b1_sb[:],
                                     gsb[:].rearrange("p c b -> p (c b)"),
                                     start=True, stop=True,
                                     skip_group_check=True)
                    rinv2 = sm.tile([4, 4 * BC], dt.bfloat16, name="ri2")
                    with nc.allow_low_precision(
                            reason="gamma direction-only renorm"):
                        nc.vector.reciprocal(rinv2[:], ss2)
                    rb2 = aux2[:]
                    nc.tensor.matmul(rb2, b1t_sb[:], rinv2[:],
                                     start=True, stop=True,
                                     skip_group_check=True)
                    gn = sm.tile([128, 4, BC], dt.bfloat16, name=f"gn{ip % 2}")
                    nc.vector.tensor_tensor(
                        gn[:], gsb[:], rb2.rearrange(
                            "p (c b) -> p c b", b=BC), AO.mult)
                    gcur, gcur_psum = gn, None

            for p in range(W2):
                y = psv.tile([128, 4 * BC], dt.float32, name="y")
                rhs = v0_sb[:].rearrange("p c b -> p (c b)") if p == 0 else \
                    vhist[:, p - 1].rearrange("p c b -> p (c b)")
                nc.tensor.matmul(y[:], e4_sb[:], rhs, start=True, stop=True)
                yv = y[:].rearrange("p (c b) -> p c b", b=BC)
                nc.vector.tensor_tensor(vhist[:, p], yv, elogv[:, :, p, :],
                                        AO.mult)
                if (p + 1) % 8 == 0:
                    r = (p + 1) // 8 - 1
                    ss = aux[0:4, :]
                    nc.tensor.matmul(
                        ss, b1_sb[:],
                        vhist[:, p].rearrange("p c b -> p (c b)"),
                        start=True, stop=True, skip_group_check=True)
                    rinv = sm.tile([4, 4 * BC], dt.bfloat16, name="rinv")
                    with nc.allow_low_precision(
                            reason="bf16 renorm scale; its exact log is "
                                   "ledgered via shist"):
                        nc.vector.reciprocal(rinv[:], ss)
                    nc.vector.tensor_copy(
                        shist[:, r], rinv[:].rearrange("p (c b) -> p c b", b=BC))
                    rb = aux2[:]
                    nc.tensor.matmul(rb, b1t_sb[:], rinv[:],
                                     start=True, stop=True,
                                     skip_group_check=True)
                    nc.vector.tensor_tensor(
                        vhist[:, p], vhist[:, p],
                        rb.rearrange("p (c b) -> p c b", b=BC), AO.mult)
                if p <= LOV:
                    left_phase(LOV - p, p)
            nc.vector.tensor_copy(gbuf[:], gcur_psum if gcur_psum is not None
                                  else gcur[:])

            # ---- handoff gather H ----
            H = sm.tile([128, 4, BC], dt.bfloat16, name="H")
            # same-pb sources at phase 31: target cg 2/3 <- cg 0/1
            nc.vector.tensor_copy(H[:, 2:4, :], vhist[:, 31, 0:2, :])
            # pb=0 targets cg 2/3 use phases 11/27 (sources s=0,1)
            nc.vector.tensor_copy(H[0:20, 2, :], vhist[0:20, 11, 0, :])
            nc.vector.tensor_copy(H[0:20, 3, :], vhist[0:20, 27, 1, :])
            # cross-pb: target (pb>=1, cg 0/1) <- (pb-1, cg+2) @31, shift +32
            # 1.0 (not 0) so untouched blocks give r=1 -> ln r = 0 (not -inf)
            HP = aux[:, 0:2 * BC].rearrange("p (c b) -> p c b", b=BC)
            nc.vector.memset(HP, 1.0)
            for pb in range(1, 4):
                for cg in range(2):
                    lhs = id_sb[32 * (pb - 1):32 * (pb - 1) + 20,
                                32 * (pb - 1):32 * (pb - 1) + 20]
                    src = vhist[32 * (pb - 1):32 * (pb - 1) + 20, 31,
                                cg + 2, :]
                    nc.tensor.matmul(
                        HP[32 * pb:32 * pb + 20, cg, :], lhs, src,
                        start=True, stop=True,
                        tile_position=(32 * (pb - 1), 32 * pb),
                        skip_group_check=True)
            nc.vector.tensor_copy(H[:, 0:2, :], HP)

            # ---- log r_s = ln(gamma.H / gamma.1) ----
            nprod = sm.tile([128, 4, BC], dt.bfloat16, name="nprod")
            nc.vector.tensor_tensor(nprod[:], gbuf[:], H[:], AO.mult)
            nm = aux[0:4, :]
            nc.tensor.matmul(nm, b1_sb[:],
                             nprod[:].rearrange("p c b -> p (c b)"),
                             start=True, stop=True, skip_group_check=True)
            dn = aux2[0:4, :]
            nc.tensor.matmul(dn, b1_sb[:],
                             gbuf[:].rearrange("p c b -> p (c b)"),
                             start=True, stop=True, skip_group_check=True)
            rin = sm.tile([4, 4 * BC], dt.float32)
            nc.vector.reciprocal(rin[:], dn)
            rr = sm.tile([4, 4, BC], dt.float32)
            nc.vector.tensor_tensor(rr[:],
                                    nm.rearrange("p (c b) -> p c b", b=BC),
                                    rin[:].rearrange("p (c b) -> p c b", b=BC),
                                    AO.mult)
            logr = sm.tile([4, 4, BC], dt.float32)
            nc.scalar.activation(logr[:], rr[:], AF.Ln)

            # ---- ledgers + lambda ----
            lnsh = sm.tile([4, NSL, 4, BC], dt.float32)
            nc.scalar.activation(lnsh[:], shist[:], AF.Ln)
            q1 = sm.tile([4, BC, 4], dt.float32)
            nc.vector.tensor_tensor(q1[:], logr[:].rearrange("p c b -> p b c"),
                                    mch_sb[:], AO.mult)
            q1r = sm.tile([4, BC], dt.float32)
            nc.vector.reduce_sum(q1r[:], q1[:], axis=mybir.AxisListType.X)
            q2 = sm.tile([4, BC, NSL, 4], dt.float32)
            nc.vector.tensor_tensor(
                q2[:], lnsh[:].rearrange("p r c b -> p b r c"), mld_sb[:],
                AO.mult)
            q2r = sm.tile([4, BC], dt.float32)
            nc.vector.reduce_sum(q2r[:], q2[:], axis=mybir.AxisListType.XY)
            lam = aux[0:1, 0:BC]
            nc.tensor.matmul(lam, o4_sb[:], q1r[:], start=True, stop=False,
                             skip_group_check=True)
            nc.tensor.matmul(lam, o4_sb[:], q2r[:], start=False, stop=True,
                             skip_group_check=True)

            # ---- extraction ----
            zp = big.tile([128, BC, W2 * 4], dt.float32)
            nc.vector.tensor_tensor(
                zp[:].rearrange("p b (w c) -> p b w c", c=4),
                vhist[:].rearrange("p w c b -> p b w c"), selz_sb[:], AO.mult)
            zr = sm.tile([128, BC], dt.float32)
            nc.vector.reduce_sum(zr[:], zp[:].rearrange(
                "p b (w c) -> p b w c", c=4), axis=mybir.AxisListType.XY)
            zs = aux2[0:1, 0:BC]
            nc.tensor.matmul(zs, o128_sb[:], zr[:], start=True, stop=True,
                             skip_group_check=True)
            lnz = sm.tile([1, BC], dt.float32)
            nc.scalar.activation(lnz[:], zs, AF.Ln)
            lamc = sm.tile([1, BC], dt.float32)
            nc.vector.tensor_copy(lamc[:], lam)
            pout = sm.tile([1, BC], dt.float32)
            nc.vector.tensor_tensor(pout[:], lnz[:], lamc[:], AO.add)
            nc.sync.dma_start(part_out[:], pout[:])
    nc.compile()
    return nc


def build_crf_program():
    bass, tile, bacc, mybir, _ = _mods()
    dt = mybir.dt
    AF = mybir.ActivationFunctionType
    AO = mybir.AluOpType

    nc = bacc.Bacc("TRN2", target_bir_lowering=False, debug=False)
    hcat = nc.dram_tensor("hcat", [4, 128, NTOK3], dt.bfloat16, kind="ExternalInput").ap()
    linw = nc.dram_tensor("linw", [4, 128, NT], dt.bfloat16, kind="ExternalInput").ap()
    linb = nc.dram_tensor("linb", [NT, 1], dt.float32, kind="ExternalInput").ap()
    etrans = nc.dram_tensor("etrans", [NT, NT], dt.float32, kind="ExternalInput").ap()
    estart = nc.dram_tensor("estart", [NT, 1], dt.float32, kind="ExternalInput").ap()
    eend = nc.dram_tensor("eend", [NT, 1], dt.float32, kind="ExternalInput").ap()
    emitmask = nc.dram_tensor("emitmask", [NT, NTOK3], dt.bfloat16, kind="ExternalInput").ap()
    lastsel = nc.dram_tensor("lastsel", [NT, BC, T], dt.bfloat16, kind="ExternalInput").ap()
    smask = nc.dram_tensor("smask", [1, BC, NREN + 1], dt.float32, kind="ExternalInput").ap()
    part_out = nc.dram_tensor("part_out", [1, BC], dt.float32, kind="ExternalOutput").ap()
    emit_out = nc.dram_tensor("emit_out", [1, 1], dt.float32, kind="ExternalOutput").ap()

    NCHUNKS3 = NTOK3 // NCH  # 4

    with tile.TileContext(nc) as tc:
        with (
            tc.tile_pool(name="w", bufs=1) as wpool,
            tc.tile_pool(name="big", bufs=1) as big,
            tc.tile_pool(name="sm", bufs=4) as sm,
            tc.tile_pool(name="pslg", bufs=2, space="PSUM") as pslg,
            tc.tile_pool(name="ps", bufs=2, space="PSUM") as ps,
        ):
            hc_sb = big.tile([128, 4, NTOK3], dt.bfloat16)
            for kc in range(4):
                nc.sync.dma_start(hc_sb[:, kc, :], hcat[kc])
            lw_sb = wpool.tile([128, 4, NT], dt.bfloat16)
            for kc in range(4):
                nc.sync.dma_start(lw_sb[:, kc, :], linw[kc])
            lb_sb = wpool.tile([NT, 1], dt.float32)
            nc.sync.dma_start(lb_sb[:], linb[:])
            et_sb = wpool.tile([NT, NT], dt.float32)
            nc.sync.dma_start(et_sb[:], etrans[:])
            es_sb = wpool.tile([NT, 1], dt.float32)
            nc.sync.dma_start(es_sb[:], estart[:])
            ee_sb = wpool.tile([NT, 1], dt.float32)
            nc.sync.dma_start(ee_sb[:], eend[:])
            em_sb = big.tile([NT, NTOK3], dt.bfloat16)
            nc.sync.dma_start(em_sb[:], emitmask[:])
            ls_sb = big.tile([NT, BC, T], dt.bfloat16)
            nc.sync.dma_start(ls_sb[:], lastsel[:])
            sm_sb = wpool.tile([1, BC, NREN + 1], dt.float32)
            nc.sync.dma_start(sm_sb[:], smask[:])
            ones_sb = wpool.tile([NT, 1], dt.float32)
            nc.vector.memset(ones_sb[:], 1.0)
            onesrow = wpool.tile([1, NT], dt.float32)
            nc.vector.memset(onesrow[:], 1.0)

            # logits^T [NT, t, b] fp32, and exp(logits)
            logits = big.tile([NT, T, BC], dt.float32)
            for n in range(NCHUNKS3):
                acc = pslg.tile([NT, NCH], dt.float32, name="lg")
                for kc in range(4):
                    nc.tensor.matmul(acc[:], lw_sb[:, kc, :],
                                     hc_sb[:, kc, n * NCH:(n + 1) * NCH],
                                     start=(kc == 0), stop=(kc == 3))
                accv = acc[:].rearrange("p (t b) -> p t b", b=BC)
                nc.vector.tensor_scalar_add(
                    logits[:, n * (NCH // BC):(n + 1) * (NCH // BC), :],
                    accv, lb_sb[:])
            elog = big.tile([NT, T, BC], dt.float32)
            nc.scalar.activation(elog[:], logits[:], AF.Exp)

            # exp-domain forward recursion, two chains of 4 sequences
            NBH = BC // 2
            shist = big.tile([1, BC, NREN + 1], dt.float32)
            nc.vector.memset(shist[:], 1.0)
            ahists = []
            for c in range(2):
                ah = big.tile([NT, NBH, T], dt.float32, name=f"ah{c}")
                nc.vector.tensor_scalar_mul(
                    ah[:, :, 0], elog[:, 0, c * NBH:(c + 1) * NBH], es_sb[:])
                ahists.append(ah)
            for t in range(1, T):
                for c in range(2):
                    ah = ahists[c]
                    bsl = slice(c * NBH, (c + 1) * NBH)
                    y = ps.tile([NT, NBH], dt.float32, name=f"y{c}", bufs=1)
                    nc.tensor.matmul(y[:], et_sb[:], ah[:, :, t - 1],
                                     start=True, stop=True)
                    if t % RENORM_EVERY == 0:
                        r = t // RENORM_EVERY - 1
                        ssum = ps.tile([NT, NBH], dt.float32, name=f"aux{c}", bufs=1)[0:1]
                        nc.tensor.matmul(ssum[:], ones_sb[:], ah[:, :, t - 1],
                                         start=True, stop=True)
                        nc.vector.tensor_copy(shist[:, bsl, r], ssum[:])
                        rinv = sm.tile([1, NBH], dt.float32, name=f"rinv{c}")
                        nc.vector.reciprocal(rinv[:], ssum[:])
                        rb = ps.tile([NT, NBH], dt.float32, name=f"aux{c}", bufs=1)
                        nc.tensor.matmul(rb[:], onesrow[:], rinv[:],
                                         start=True, stop=True)
                        u1 = sm.tile([NT, NBH], dt.float32, name=f"u1{c}")
                        nc.vector.tensor_tensor(u1[:], y[:], elog[:, t, bsl],
                                                AO.mult)
                        nc.vector.tensor_tensor(ah[:, :, t], u1[:], rb[:],
                                                AO.mult)
                    else:
                        nc.vector.tensor_tensor(ah[:, :, t], y[:],
                                                elog[:, t, bsl], AO.mult)

            # partition_b = ln(sum_j a[len_b-1, j] * e_end[j]) + sum_r ln(s_rb)
            alast = sm.tile([NT, BC], dt.float32)
            for c in range(2):
                bsl = slice(c * NBH, (c + 1) * NBH)
                prod = big.tile([NT, NBH, T], dt.float32, name=f"prod{c}")
                nc.vector.tensor_tensor(prod[:], ahists[c][:], ls_sb[:, bsl, :],
                                        AO.mult)
                nc.vector.reduce_sum(alast[:, bsl], prod[:],
                                     axis=mybir.AxisListType.X)
            w2 = sm.tile([NT, BC], dt.float32)
            nc.vector.tensor_scalar_mul(w2[:], alast[:], ee_sb[:])
            fsum = ps.tile([1, BC], dt.float32, name="faux", bufs=1)
            nc.tensor.matmul(fsum[:], ones_sb[:], w2[:], start=True, stop=True)
            pln = sm.tile([1, BC], dt.float32)
            nc.scalar.activation(pln[:], fsum[:], AF.Ln)
            slog = sm.tile([1, BC, NREN + 1], dt.float32)
            nc.scalar.activation(slog[:], shist[:], AF.Ln)
            slogm = sm.tile([1, BC, NREN + 1], dt.float32)
            nc.vector.tensor_tensor(slogm[:], slog[:], sm_sb[:], AO.mult)
            zb = sm.tile([1, BC], dt.float32)
            nc.vector.reduce_sum(zb[:], slogm[:], axis=mybir.AxisListType.X)
            pout = sm.tile([1, BC], dt.float32)
            nc.vector.tensor_tensor(pout[:], pln[:], zb[:], AO.add)
            nc.sync.dma_start(part_out[:], pout[:])

            # emission score total
            eprod = big.tile([NT, T, BC], dt.float32)
            nc.vector.tensor_tensor(
                eprod[:], logits[:],
                em_sb[:].rearrange("p (t b) -> p t b", b=BC), AO.mult)
            erow = sm.tile([NT, 1], dt.float32)
            nc.vector.reduce_sum(erow[:], eprod[:], axis=mybir.AxisListType.XY)
            etot = ps.tile([1, 1], dt.float32, name="faux", bufs=1)
            nc.tensor.matmul(etot[:], ones_sb[:], erow[:], start=True, stop=True)
            eout = sm.tile([1, 1], dt.float32)
            nc.vector.tensor_copy(eout[:], etot[:])
            nc.sync.dma_start(emit_out[:], eout[:])
    nc.compile()
    return nc


# --------------------------------------------------------------------------
# host-side data prep
# --------------------------------------------------------------------------

def _crf_v2_consts(trans, start_t, end_t, lin_b):
    """Input tensors shared by all cores for the v2 CRF launch."""
    E = np.exp(trans.astype(np.float64))
    e4 = np.zeros((128, 128), np.float32)
    e4t = np.zeros((128, 128), np.float32)
    for pb in range(4):
        e4[32 * pb:32 * pb + 20, 32 * pb:32 * pb + 20] = E
        e4t[32 * pb:32 * pb + 20, 32 * pb:32 * pb + 20] = E.T
    alpha_m1 = np.linalg.solve(E.T, np.exp(start_t.astype(np.float64)))
    v0 = np.zeros((128, 4, BC), np.float32)
    g0 = np.zeros((128, 4, BC), np.float32)
    for pb in range(4):
        for cg in range(4):
            s = pb * 4 + cg
            init = alpha_m1 if s < 2 else np.ones(NT)
            v0[32 * pb:32 * pb + 20, cg, :] = init[:, None]
            g0[32 * pb:32 * pb + 20, cg, :] = 1.0
    b1 = np.zeros((128, 4), np.float32)
    b1t = np.zeros((4, 128), np.float32)
    for pb in range(4):
        b1[32 * pb:32 * pb + 20, pb] = 1.0
        b1t[pb, 32 * pb:32 * pb + 20] = 1.0
    identb = np.eye(128, dtype=np.float32)
    linb4 = np.zeros((128, 1), np.float32)
    for pb in range(4):
        linb4[32 * pb:32 * pb + 20, 0] = lin_b
    return {
        "e4": e4.astype(BF16), "e4t": e4t.astype(BF16),
        "v0": v0.astype(BF16), "g0": g0.astype(BF16),
        "b1": b1.astype(BF16), "b1t": b1t.astype(BF16),
        "identb": identb.astype(BF16), "linb4": linb4,
        "ones4": np.ones((4, 1), np.float32),
        "ones128": np.ones((128, 1), np.float32),
    }


def _crf_v2_seq_masks(lens_c, labels_c, end_t):
    """selz / emmask / mchain / mled for one core's BC sequences."""
    ee = np.exp(end_t.astype(np.float64)).astype(np.float32)
    selz = np.zeros((128, BC, W2, 4), np.float32)
    emmask = np.zeros((128, 4, W2, BC), np.float32)
    mchain = np.zeros((4, BC, 4), np.float32)
    mled = np.zeros((4, BC, NSL, 4), np.float32)
    for b in range(BC):
        L = int(lens_c[b]) - 1
        sb = min(L // LSEG, S2 - 1)
        pph = L - WST[sb]
        pb_b, cg_b = divmod(sb, 4)
        selz[32 * pb_b:32 * pb_b + 20, b, pph, cg_b] = ee
        # extraction renorm ledger
        for r in range(NSL):
            if pph >= 8 * r + 7:
                mled[pb_b, b, r, cg_b] += 1.0
        # chain
        s = sb
        while s >= 2:
            pbs, cgs = divmod(s, 4)
            mchain[pbs, b, cgs] = 1.0
            sp, pp = SRC_SLOT[s]
            pbp, cgp = divmod(sp, 4)
            for r in range(NSL):
                if pp >= 8 * r + 7:
                    mled[pbp, b, r, cgp] += 1.0
            s = sp
        # emission ownership
        for t in range(int(lens_c[b])):
            s = t // LSEG
            pbs, cgs = divmod(s, 4)
            p = t - WST[s]
            emmask[32 * pbs + int(labels_c[b, t]), cgs, p, b] = 1.0
    # shist holds the bf16 reciprocal actually applied, so its log enters
    # the ledger with the opposite sign
    return (np.ascontiguousarray(selz).astype(BF16),
            np.ascontiguousarray(emmask.reshape(128, 4, W2 * BC)).astype(BF16),
            mchain, -mled)


def _layer_inputs(xin, w_ih, w_hh, b_ih, b_hh):
    """Per-core input dicts for one layer launch.

    xin: [2, B, T, K] fp32 (xin[1] already reversed+masked)
    w_ih: [2, 4HD, K]; w_hh: [2, 4HD, HD]; b_ih, b_hh: [2, 4HD]
    """
    K = xin.shape[-1]
    kc_in = K // 128
    # scale the g-gate rows (post-perm block 3) by 2: tanh(x) = 2*sig(2x)-1
    gscale = np.ones((4 * HD, 1), np.float32)
    gscale[2 * HD:3 * HD] = 2.0
    per_dir = []
    for d in range(2):
        wih_p = w_ih[d][_PERM] * gscale
        whh_p = w_hh[d][_PERM] * gscale
        b_p = (b_ih[d] + b_hh[d])[_PERM] * gscale[:, 0]
        wihT = np.ascontiguousarray(
            wih_p.T.reshape(kc_in, 128, 4 * HD)).astype(BF16)
        whhT = np.ascontiguousarray(
            whh_p.T.reshape(2, 128, 4 * HD)).astype(BF16)
        bs = np.ascontiguousarray(
            b_p.reshape(NJ, 128).T).astype(np.float32)
        per_dir.append((wihT, whhT, bs))
    maps = []
    for core in range(NCORES):
        d, q = divmod(core, 4)
        xc = xin[d, q * BL:(q + 1) * BL]              # [BL, T, K]
        xT = np.ascontiguousarray(
            xc.transpose(2, 1, 0).reshape(kc_in, 128, T * BL)).astype(BF16)
        wihT, whhT, bs = per_dir[d]
        maps.append({"xT": xT, "wih": wihT, "whh": whhT, "bias": bs})
    return maps


def _collect_h(results):
    """per-core 'hout' [128,2,T,BL] bf16 -> h [2, B, T, HD] fp32."""
    h = np.empty((2, B, T, HD), np.float32)
    for core in range(NCORES):
        d, q = divmod(core, 4)
        ho = np.asarray(results[core]["hout"], dtype=np.float32)
        h[d, q * BL:(q + 1) * BL] = ho.transpose(3, 2, 1, 0).reshape(BL, T, HD)
    return h


def _unreverse(h_rev, lens, valid):
    """h_rev[b, s] holds position lens_b-1-s; return h[b, t] (zeros at pad)."""
    t = np.arange(T)
    idx = np.clip(lens[:, None] - 1 - t[None, :], 0, T - 1)
    out = np.take_along_axis(h_rev, idx[:, :, None], axis=1)
    return out * valid[:, :, None]


def kernel(**inputs):
    _, _, _, _, run_bass_kernel_spmd = _mods()
    global LAST_RESULTS
    LAST_RESULTS = []
    trace = bool(int(os.environ.get("KERNEL_TRACE", "0")))
    if trace:
        _install_ntff_shim()

    tokens = np.asarray(inputs["tokens"]).astype(np.int64)
    lens = np.asarray(inputs["lens"]).astype(np.int64)
    labels = np.asarray(inputs["labels"]).astype(np.int64)
    emb = np.asarray(inputs["emb"], dtype=np.float32)
    w_ih = [np.asarray(inputs["w_ih_l0"], np.float32),
            np.asarray(inputs["w_ih_l1"], np.float32)]
    w_hh = [np.asarray(inputs["w_hh_l0"], np.float32),
            np.asarray(inputs["w_hh_l1"], np.float32)]
    b_ih = [np.asarray(inputs["b_ih_l0"], np.float32),
            np.asarray(inputs["b_ih_l1"], np.float32)]
    b_hh = [np.asarray(inputs["b_hh_l0"], np.float32),
            np.asarray(inputs["b_hh_l1"], np.float32)]
    lin_w = np.asarray(inputs["lin_w"], np.float32)
    lin_b = np.asarray(inputs["lin_b"], np.float32)
    trans = np.asarray(inputs["trans"], np.float32)
    start_t = np.asarray(inputs["start_t"], np.float32)
    end_t = np.asarray(inputs["end_t"], np.float32)

    t_ar = np.arange(T)
    valid = (t_ar[None, :] < lens[:, None]).astype(np.float32)
    rev_idx = np.clip(lens[:, None] - 1 - t_ar[None, :], 0, T - 1)

    if "layer0" not in _CACHE:
        _CACHE["layer0"] = build_layer_program(E // 128)
    if "layer1" not in _CACHE:
        _CACHE["layer1"] = build_layer_program(2 * HD // 128)
    if "crf2" not in _CACHE:
        _CACHE["crf2"] = build_crf_program_v2()

    cores = list(range(NCORES))

    # ---------- launch 1: layer 0 ----------
    x = emb[tokens]
    x_rev = np.take_along_axis(x, rev_idx[:, :, None], axis=1) * valid[:, :, None]
    xin0 = np.stack([x, x_rev])
    res1 = run_bass_kernel_spmd(
        _CACHE["layer0"], _layer_inputs(xin0, w_ih[0], w_hh[0], b_ih[0], b_hh[0]),
        cores, trace=trace)
    LAST_RESULTS.append(res1)
    h0 = _collect_h(res1.results)

    # ---------- launch 2: layer 1 ----------
    h0f = h0[0] * valid[:, :, None]
    h0b = _unreverse(h0[1], lens, valid)
    x1 = np.concatenate([h0f, h0b], axis=-1)
    x1_rev = np.take_along_axis(x1, rev_idx[:, :, None], axis=1) * valid[:, :, None]
    xin1 = np.stack([x1, x1_rev])
    res2 = run_bass_kernel_spmd(
        _CACHE["layer1"], _layer_inputs(xin1, w_ih[1], w_hh[1], b_ih[1], b_hh[1]),
        cores, trace=trace)
    LAST_RESULTS.append(res2)
    h1 = _collect_h(res2.results)

    # ---------- launch 3: logits + CRF ----------
    h1f = h1[0] * valid[:, :, None]
    h1b = _unreverse(h1[1], lens, valid)
    hcat = np.concatenate([h1f, h1b], axis=-1)

    lw = np.ascontiguousarray(lin_w.T.reshape(4, 128, NT)).astype(BF16)
    consts = _crf_v2_consts(trans, start_t, end_t, lin_b)
    maps = []
    for core in range(NCORES):
        bs = slice(core * BC, (core + 1) * BC)
        hc = hcat[bs]
        hcT = np.ascontiguousarray(
            hc.transpose(2, 1, 0).reshape(4, 128, T * BC)).astype(BF16)
        selz, emmask, mchain, mled = _crf_v2_seq_masks(
            lens[bs], labels[bs], end_t)
        m = {"hcat": hcT, "linw": lw, "selz": selz, "emmask": emmask,
             "mchain": mchain, "mled": mled}
        m.update(consts)
        maps.append(m)
    res3 = run_bass_kernel_spmd(_CACHE["crf2"], maps, cores, trace=trace)
    LAST_RESULTS.append(res3)

    partition = np.concatenate(
        [np.asarray(r["part_out"])[0] for r in res3.results])
    emit = float(sum(np.asarray(r["emit_out"])[0, 0] for r in res3.results))

    # host-side numerator terms (incl. the logits-bias part of the emission
    # score: the device emission uses bias-free logits)
    first_tag = labels[:, 0]
    last_tag = np.take_along_axis(labels, (lens - 1)[:, None], axis=1)[:, 0]
    tr_sc = float((trans[labels[:, :-1], labels[:, 1:]] * valid[:, 1:]).sum())
    emit_b = float((lin_b[labels] * valid).sum())
    host_num = (float(start_t[first_tag].sum()) + tr_sc
                + float(end_t[last_tag].sum()) + emit_b)

    loss = partition.sum() - emit - host_num
    return np.float32(loss)



# revision 32
# speedup vs baseline: 1.0255x; 1.0255x over previous
"""BiLSTM-CRF NLL kernel for 8 Trainium2 NeuronCores.

Strategy (3 SPMD launches, host glue between them):
  L1 "layer0": 8 cores = 2 dirs x 4 batch-quarters (16 seqs/core, one LSTM dir).
     Per core: gx = W_ih @ x^T (+biases) as chunked matmuls interleaved with the
     256-step recurrent scan (weight-stationary matmuls; gates in a
     [128-partition, batch-free] layout so elementwise ops use all lanes).
     The per-step serial chain is minimized with two runtime-registered custom
     DVE ops: ct = clamp(w+m) and h = sig_o*tanh_poly(2*ct), which removes the
     tanh ACTIVATE, two tensor_tensor ops, and their cross-engine syncs from
     the recurrence-critical path.
  L2 "layer1": same program shape with K=512 input; host reshards and handles
     the per-sequence reversal of the backward direction.
  L3 "logits+CRF": 8 cores = 8 batch-eighths. The 255-step serial CRF scan is
     replaced by 16 overlapped 36-phase window scans (4 partition-blocks x 4
     col-groups, block-diagonal exp(trans) matmuls): products of the positive
     transition operator contract to rank-1 in ~20 steps (Birkhoff), so each
     window recovers the true alpha direction after its 20-step overlap, and
     per-segment log-scales chain through left-vector (gamma window scan)
     inner products. Extraction at t=len-1, renorm ledgers and the scale chain
     are folded into host-built masks. start/end/transition/bias numerator
     terms are summed on host.

Matmuls run in bf16 (fp32 PSUM accumulate); LSTM cell state is fp32.
"""

import os
import sys

import numpy as np

for _p in ("/opt/trn_rl_repo", "/root/.axon_site/_ro/trn_rl_repo"):
    if _p not in sys.path and os.path.isdir(_p):
        sys.path.insert(0, _p)

import ml_dtypes  # noqa: E402

BF16 = ml_dtypes.bfloat16

B, T, V, E, HD, NT = 64, 256, 50000, 256, 256, 20
NCORES = 8
BL = 16            # sequences per core in L1/L2 (batch quarter)
BC = 8             # sequences per core in L3 (batch eighth)
NTOK = BL * T      # tokens per core in L1/L2
NTOK3 = BC * T     # tokens per core in L3
NJ = 8             # gate tiles of 128 rows (4 gates x 256 HD / 128)
NCH = 512          # matmul N-chunk (tokens)
TCH = NCH // BL    # timesteps per gx chunk (32)
RENORM_EVERY = 8   # CRF renormalization interval
NREN = (T - 1) // RENORM_EVERY   # renorm slots used (t = 8,16,...,248)

# gate order stays pytorch-native (i,f,g,o): the c-path gates (i,f,g) are
# tiles 0..5 (one contiguous sigmoid), o is tiles 6..7 (deferred off the
# critical path). g rows are pre-scaled by 2 so tanh(x) = 2*sig(2x)-1.
_PERM = np.arange(4 * HD)

# cell state kept as ct = c/2 so ct = sig_f*ct + w (w = sig_i*(sig(2g)-0.5));
# tanh(c) = tanh(2*ct) evaluated as a clamped odd quintic on the DVE, which
# removes the tanh ACTIVATE (and a cross-engine sync) from the serial chain.
CLAMP_B = 1.25                    # ct clamp (real c clamped at 2*CLAMP_B)
# odd-quintic minimax fit of tanh(2x) on [0, CLAMP_B] (see _fit_tanh2)
TANH5_COEF = None                 # filled by _fit_tanh2() at import
USE_TH3 = True                    # fuse h = sig_o*tanh (deg-3) into one op

_CACHE = {}
LAST_RESULTS = []   # BassKernelResults of the launches of the last kernel() call


def _fit_tanh2(Bc):
    x = np.linspace(1e-6, Bc, 4001, dtype=np.float64)
    t = np.tanh(2.0 * x)
    q = x * x
    A = np.stack([x, x * q, x * q * q], 1)
    w = np.ones_like(x)
    sol = None
    for _ in range(80):
        sol, *_ = np.linalg.lstsq(A * w[:, None], t * w, rcond=None)
        err = A @ sol - t
        w = 1.0 + 10.0 * np.abs(err) / np.abs(err).max()
    return [float(v) for v in sol]


TANH5_COEF = _fit_tanh2(CLAMP_B)


def _fit_tanh2_deg3(Bc):
    x = np.linspace(1e-6, Bc, 4001, dtype=np.float64)
    t = np.tanh(2.0 * x)
    q = x * x
    A = np.stack([x, x * q], 1)
    w = np.ones_like(x)
    sol = None
    for _ in range(100):
        sol, *_ = np.linalg.lstsq(A * w[:, None], t * w, rcond=None)
        err = A @ sol - t
        w = 1.0 + 12.0 * np.abs(err) / np.abs(err).max()
    return [float(v) for v in sol]


TANH3_COEF = _fit_tanh2_deg3(CLAMP_B)

_DVE_OPS = {}


def _register_dve_ops():
    """Runtime-register the two fused cell ops with the custom-DVE table."""
    if _DVE_OPS:
        return _DVE_OPS
    from concourse import dve_ops as DVO
    from concourse.dve_spec import (
        Spec, Src0, Src1, C0, C1, C2, Zero, maxx, minn, lower)
    from concourse.dve_spec import _has_src1 as has_src1
    from concourse.dve_uop import DveOpSpec

    def make(name, spec):
        if name in DVO.CUSTOM_DVE_SPECS:
            return next(op for op in DVO.OPS if op.name == name)
        row = DVO._CUSTOM_DVE_ROW_BASE + len(DVO.OPS)
        DVO._SUB_OPCODE_FOR_NAME[name] = row
        shas = {}
        for ver in ("v3", "v4"):
            uops = lower(spec, ver=ver)
            shas[ver] = DveOpSpec(name=name, opcode=row, uops=uops,
                                  rd1_en=has_src1(spec)).sha(ver)
        op = DVO.DveOp(name, spec, subdim=False, uops_sha=shas)
        DVO.OPS.append(op)
        DVO.CUSTOM_DVE_SPECS[name] = spec
        return op

    # ct = clamp(w + m, -C0, C0)
    s = Src0 + Src1
    body_ca = minn(maxx(s, Zero - C0), C0)
    clampadd = make("ANT_CLAMPADD", Spec(
        body=body_ca,
        reference=lambda in0, in1, s0: np.clip(
            in0.astype(np.float32) + in1.astype(np.float32), -s0, s0),
    ))

    # out = x*(C0 + C1*x^2 + C2*x^4)   (odd quintic; input pre-clamped)
    q = Src0 * Src0
    body_t5 = Src0 * ((C2 * q + C1) * q + C0)
    tanh5 = make("ANT_TANH5", Spec(
        body=body_t5,
        reference=lambda in0, s0, s1, imm2: (
            in0 * (s0 + s1 * in0 * in0 + imm2 * (in0 * in0) ** 2)
        ).astype(np.float32),
    ))
    # h = sig_o * tanh3(ct): odd cubic with the gate product fused
    q3 = Src0 * Src0
    body_t3 = Src1 * (Src0 * (C1 * q3 + C0))
    th3mul = make("ANT_TH3MUL", Spec(
        body=body_t3,
        reference=lambda in0, in1, s0, s1: (
            in1 * in0 * (s0 + s1 * in0 * in0)).astype(np.float32),
    ))
    _DVE_OPS["clampadd"] = clampadd
    _DVE_OPS["tanh5"] = tanh5
    _DVE_OPS["th3mul"] = th3mul
    return _DVE_OPS


def _mods():
    import concourse.bass as bass
    import concourse.tile as tile
    from concourse import bacc, mybir
    from concourse.bass_utils import run_bass_kernel_spmd
    return bass, tile, bacc, mybir, run_bass_kernel_spmd


def _install_ntff_shim():
    """Provide antenv.axon_hooks (missing in this image) so that
    run_bass_kernel_spmd(trace=True) can capture NTFF profiles through
    libaxon_pjrt.so. Mirrors trn_agent_boot._ntff_profile_via_ctypes."""
    import sys as _sys
    if "antenv.axon_hooks" in _sys.modules:
        return
    import contextlib
    import ctypes
    import types

    so_path = "/opt/axon/libaxon_pjrt.so"
    mod = types.ModuleType("antenv.axon_hooks")
    _hook_box = [None]

    def set_axon_ntff_profile_hook(h):
        _hook_box[0] = h

    def get_axon_ntff_profile_hook():
        return _hook_box[0]

    mod.set_axon_ntff_profile_hook = set_axon_ntff_profile_hook
    mod.get_axon_ntff_profile_hook = get_axon_ntff_profile_hook
    _sys.modules["antenv.axon_hooks"] = mod

    try:
        lib = ctypes.CDLL(so_path)
        if not hasattr(lib, "axon_start_nrt_profile"):
            return
        lib.axon_start_nrt_profile.argtypes = [
            ctypes.POINTER(ctypes.c_int64), ctypes.c_size_t]
        lib.axon_start_nrt_profile.restype = ctypes.c_int64
        lib.axon_stop_nrt_profile.argtypes = [ctypes.c_char_p]
        lib.axon_stop_nrt_profile.restype = ctypes.c_int64

        @contextlib.contextmanager
        def _hook(output_dir, device_ids):
            import jax
            jax.devices()
            if device_ids:
                ids = (ctypes.c_int64 * len(device_ids))(*device_ids)
                rc = lib.axon_start_nrt_profile(ids, len(device_ids))
            else:
                rc = lib.axon_start_nrt_profile(None, 0)
            if rc != 0:
                raise RuntimeError(f"axon_start_nrt_profile rc={rc}")
            try:
                yield
            finally:
                n = lib.axon_stop_nrt_profile(str(output_dir).encode())
                print(f"profile: {n} file(s) written to {output_dir}",
                      file=sys.stderr)

        set_axon_ntff_profile_hook(_hook)
    except OSError:
        pass


# --------------------------------------------------------------------------
# program builders
# --------------------------------------------------------------------------

def build_layer_program(kc_in):
    """One BiLSTM direction for BL sequences. kc_in = input dim / 128."""
    bass, tile, bacc, mybir, _ = _mods()
    dt = mybir.dt
    AF = mybir.ActivationFunctionType
    AO = mybir.AluOpType
    dve = _register_dve_ops()
    a0, a1, a2 = TANH5_COEF

    nc = bacc.Bacc("TRN2", target_bir_lowering=False, debug=False)
    xT = nc.dram_tensor("xT", [kc_in, 128, NTOK], dt.bfloat16, kind="ExternalInput").ap()
    wih = nc.dram_tensor("wih", [kc_in, 128, 4 * HD], dt.bfloat16, kind="ExternalInput").ap()
    whh = nc.dram_tensor("whh", [2, 128, 4 * HD], dt.bfloat16, kind="ExternalInput").ap()
    bias = nc.dram_tensor("bias", [128, NJ], dt.float32, kind="ExternalInput").ap()
    hout = nc.dram_tensor("hout", [128, 2, T, BL], dt.bfloat16,
                          kind="ExternalOutput").ap()

    NCHUNKS = NTOK // NCH  # 8

    with tile.TileContext(nc) as tc:
        with (
            tc.tile_pool(name="w", bufs=1) as wpool,
            tc.tile_pool(name="big", bufs=1) as big,
            tc.tile_pool(name="gxp", bufs=2) as gxp,
            tc.tile_pool(name="xs", bufs=3) as xs,
            tc.tile_pool(name="st", bufs=1) as st,
            tc.tile_pool(name="ew", bufs=4) as ew,
            tc.tile_pool(name="ps", bufs=2, space="PSUM") as ps,
            tc.tile_pool(name="psg", bufs=3, space="PSUM") as psg,
        ):
            wih_sb = wpool.tile([128, kc_in, 4 * HD], dt.bfloat16)
            whh_sb = wpool.tile([128, 2, 4 * HD], dt.bfloat16)
            bias_sb = wpool.tile([128, NJ], dt.float32)
            # one large DMA per weight chunk: a per-j-slice split (to start gx
            # earlier) measured 29us slower overall — small 256B-line DMAs
            # lose more bandwidth than the earlier start saves
            for kc in range(kc_in):
                nc.sync.dma_start(wih_sb[:, kc, :], wih[kc])
            for kc in range(2):
                nc.sync.dma_start(whh_sb[:, kc, :], whh[kc])
            nc.sync.dma_start(bias_sb[:], bias[:])
            from concourse.masks import make_identity
            ident = wpool.tile([128, 128], dt.bfloat16)
            make_identity(nc, ident[:])

            hist = big.tile([128, 2, T + 1, BL], dt.bfloat16)
            cst = st.tile([128, 2, BL], dt.float32)
            nc.vector.memset(hist[:, :, 0, :], 0.0)
            nc.vector.memset(cst[:], 0.0)

            # gx compute for chunk n, interleaved into chunk n-1's scan steps
            def gx_block(gxb, xc, j):
                acc = psg.tile([128, NCH], dt.float32, name="acc")
                for kc in range(kc_in):
                    nc.tensor.matmul(
                        acc[:], wih_sb[:, kc, j * 128:(j + 1) * 128],
                        xc[:, kc, :],
                        start=(kc == 0), stop=(kc == kc_in - 1))
                accv = acc[:].rearrange("p (t b) -> p t b", b=BL)
                # scalar engine: with tanh moved to the DVE the scalar engine
                # has slack, and the DVE FIFO is now the per-step chain pole
                nc.scalar.add(gxb[:, j], accv, bias_sb[:, j:j + 1])

            def load_x(n):
                xc = xs.tile([128, kc_in, NCH], dt.bfloat16, name="xc")
                for kc in range(kc_in):
                    nc.sync.dma_start(xc[:, kc, :],
                                      xT[kc, :, n * NCH:(n + 1) * NCH])
                return xc

            def prefill(gxb, tt):
                # identity matmuls drop gx(+bias) for a whole step into PSUM;
                # o-gates go to their own bank so the c-path sigmoid is not
                # gated on them (PSUM deps are bank-granular)
                G1 = ps.tile([128, 6, BL], dt.float32, name="G1")
                nc.tensor.matmul(G1[:], ident[:], gxb[:, 0:6, tt, :],
                                 start=True, stop=False, skip_group_check=True)
                G2 = ps.tile([128, 2, BL], dt.float32, name="G2")
                nc.tensor.matmul(G2[:], ident[:], gxb[:, 6:8, tt, :],
                                 start=True, stop=False, skip_group_check=True)
                return G1, G2

            # chunk 0 gx up front
            xc_cur = load_x(0)
            gx_cur = gxp.tile([128, NJ, TCH, BL], dt.bfloat16, name="gxb")
            for j in range(NJ):
                gx_block(gx_cur, xc_cur, j)
            Gc = prefill(gx_cur, 0)

            # scan; cell (g rows pre-scaled by 2 on host):
            #   w = (sig_g' - 0.5) * sig_i ; c = 2w + sig_f*c ; h = sig_o*tanh(c)
            for n in range(NCHUNKS):
                gx_nxt = None
                if n + 1 < NCHUNKS:
                    xc_nxt = load_x(n + 1)
                    gx_nxt = gxp.tile([128, NJ, TCH, BL], dt.bfloat16,
                                      name="gxb")
                for tt in range(TCH):
                    t = n * TCH + tt
                    G1, G2 = Gc
                    # c-path gates (i,f,g) first; kc-major so the kc=0 MMs can
                    # start as soon as the kc=0 half of h(t-1) is written
                    for kc in range(2):
                        for j in range(6):
                            nc.tensor.matmul(
                                G1[:, j, :], whh_sb[:, kc, j * 128:(j + 1) * 128],
                                hist[:, kc, t, :], start=False,
                                stop=(j == 5 and kc == 1),
                                skip_group_check=True)
                    # o-gate matmuls + its sigmoid run off the critical path
                    for kc in range(2):
                        for j in (6, 7):
                            nc.tensor.matmul(
                                G2[:, j - 6, :],
                                whh_sb[:, kc, j * 128:(j + 1) * 128],
                                hist[:, kc, t, :], start=False,
                                stop=(j == 7 and kc == 1),
                                skip_group_check=True)
                    # prefill next step's PSUM + spread next chunk's gx matmuls
                    if tt + 1 < TCH:
                        Gc = prefill(gx_cur, tt + 1)
                    elif gx_nxt is not None:
                        Gc = prefill(gx_nxt, 0)
                    if gx_nxt is not None and tt % 4 == 1 and tt // 4 < NJ:
                        gx_block(gx_nxt, xc_nxt, tt // 4)

                    A1 = ew.tile([128, 6, BL], dt.float32, name="A1")
                    nc.scalar.activation(A1[:], G1[:], AF.Sigmoid)
                    A2 = ew.tile([128, 2, BL], dt.float32, name="A2")
                    nc.scalar.activation(A2[:], G2[:], AF.Sigmoid)
                    w = ew.tile([128, 2, BL], dt.float32, name="w")
                    nc.vector.scalar_tensor_tensor(
                        w[:], A1[:, 4:6, :], 0.5, A1[:, 0:2, :],
                        AO.subtract, AO.mult)
                    m1 = ew.tile([128, 2, BL], dt.float32, name="m1")
                    nc.vector.tensor_tensor(m1[:], A1[:, 2:4, :], cst[:],
                                            AO.mult)
                    # ct = clamp(w + m, +-B); tanh(2*ct) as odd polynomial and
                    # the sig_o product fused on the DVE — no tanh ACTIVATE or
                    # extra sync on the chain
                    nc.vector._custom_dve(dve["clampadd"], out=cst[:],
                                          in0=w[:], in1=m1[:], s0=CLAMP_B)
                    if USE_TH3:
                        b0, b1c = TANH3_COEF
                        # single fused h write: a kc-split variant (earlier
                        # kc=0 matmul start) measured 29us slower overall —
                        # the extra DVE op's fixed cost outweighs the overlap
                        nc.vector._custom_dve(
                            dve["th3mul"], out=hist[:, :, t + 1, :],
                            in0=cst[:], in1=A2[:], s0=b0, s1=b1c)
                    else:
                        Tc = ew.tile([128, 2, BL], dt.float32, name="Tc")
                        nc.vector._custom_dve(dve["tanh5"], out=Tc[:],
                                              in0=cst[:], s0=a0, s1=a1,
                                              imm2=a2)
                        nc.vector.tensor_tensor(hist[:, 0, t + 1, :],
                                                A2[:, 0, :], Tc[:, 0, :],
                                                AO.mult)
                        nc.vector.tensor_tensor(hist[:, 1, t + 1, :],
                                                A2[:, 1, :], Tc[:, 1, :],
                                                AO.mult)
                if gx_nxt is not None:
                    gx_cur, xc_cur = gx_nxt, xc_nxt
                # stream finished history out
                t0 = n * TCH
                nc.sync.dma_start(hout[:, :, t0:t0 + TCH, :],
                                  hist[:, :, t0 + 1:t0 + TCH + 1, :])
    nc.compile()
    return nc


# ---- CRF v2: rank-1 overlapped-segment scan -------------------------------
# 16 time-segments of 16 steps, laid out as 4 partition-blocks (base 0/32/64/
# 96) x 4 col-groups. Each segment scans a 36-phase window (20-step overlap
# into its predecessor) from a uniform init; Birkhoff contraction makes the
# windowed transfer operator rank-1 (~0.42/step), so the true alpha direction
# is recovered after the overlap and per-segment log-scale factors chain via
# left-vector inner products (gamma scans over the overlap windows).
S2, LSEG, W2, LOV = 16, 16, 36, 20
NSL = 4                      # renorm ledger slots (after phases 7/15/23/31)
WST = [0 if s < 2 else 16 * s - LOV for s in range(S2)]
SRC_SLOT = {s: ((0, 11) if s == 2 else (1, 27) if s == 3 else (s - 2, 31))
            for s in range(2, S2)}


def build_crf_program_v2():
    bass, tile, bacc, mybir, _ = _mods()
    dt = mybir.dt
    AF = mybir.ActivationFunctionType
    AO = mybir.AluOpType

    nc = bacc.Bacc("TRN2", target_bir_lowering=False, debug=False)
    hcat = nc.dram_tensor("hcat", [4, 128, NTOK3], dt.bfloat16, kind="ExternalInput").ap()
    linw = nc.dram_tensor("linw", [4, 128, NT], dt.bfloat16, kind="ExternalInput").ap()
    linb4 = nc.dram_tensor("linb4", [128, 1], dt.float32, kind="ExternalInput").ap()
    e4 = nc.dram_tensor("e4", [128, 128], dt.bfloat16, kind="ExternalInput").ap()
    e4t = nc.dram_tensor("e4t", [128, 128], dt.bfloat16, kind="ExternalInput").ap()
    v0 = nc.dram_tensor("v0", [128, 4, BC], dt.bfloat16, kind="ExternalInput").ap()
    g0 = nc.dram_tensor("g0", [128, 4, BC], dt.bfloat16, kind="ExternalInput").ap()
    b1 = nc.dram_tensor("b1", [128, 4], dt.bfloat16, kind="ExternalInput").ap()
    b1t = nc.dram_tensor("b1t", [4, 128], dt.bfloat16, kind="ExternalInput").ap()
    ident = nc.dram_tensor("identb", [128, 128], dt.bfloat16, kind="ExternalInput").ap()
    selz = nc.dram_tensor("selz", [128, BC, W2, 4], dt.bfloat16, kind="ExternalInput").ap()
    emmask = nc.dram_tensor("emmask", [128, 4, W2 * BC], dt.bfloat16, kind="ExternalInput").ap()
    mchain = nc.dram_tensor("mchain", [4, BC, 4], dt.float32, kind="ExternalInput").ap()
    mled = nc.dram_tensor("mled", [4, BC, NSL, 4], dt.float32, kind="ExternalInput").ap()
    ones4 = nc.dram_tensor("ones4", [4, 1], dt.float32, kind="ExternalInput").ap()
    ones128 = nc.dram_tensor("ones128", [128, 1], dt.float32, kind="ExternalInput").ap()
    part_out = nc.dram_tensor("part_out", [1, BC], dt.float32, kind="ExternalOutput").ap()
    emit_out = nc.dram_tensor("emit_out", [1, 1], dt.float32, kind="ExternalOutput").ap()

    WB = W2 * BC  # 288 cols per (pb, cg) logits slab

    with tile.TileContext(nc) as tc:
        with (
            tc.tile_pool(name="w", bufs=1) as wp,
            tc.tile_pool(name="big", bufs=1) as big,
            tc.tile_pool(name="sm", bufs=4) as sm,
            tc.tile_pool(name="pslg", bufs=2, space="PSUM") as pslg,
            tc.tile_pool(name="psv", bufs=2, space="PSUM") as psv,
            tc.tile_pool(name="psg", bufs=2, space="PSUM") as psg,
            tc.tile_pool(name="psx", bufs=1, space="PSUM") as psx,
        ):
            # two shared PSUM scratch tiles (bank-quantized: 8 banks total)
            aux = psx.tile([128, 4 * BC], dt.float32, name="aux")
            aux2 = psx.tile([128, 4 * BC], dt.float32, name="aux2")
            hc_sb = big.tile([128, 4, NTOK3], dt.bfloat16)
            for kc in range(4):
                nc.sync.dma_start(hc_sb[:, kc, :], hcat[kc])
            lw_sb = wp.tile([128, 4, NT], dt.bfloat16)
            for kc in range(4):
                nc.sync.dma_start(lw_sb[:, kc, :], linw[kc])
            lb_sb = wp.tile([128, 1], dt.float32)
            nc.sync.dma_start(lb_sb[:], linb4[:])
            e4_sb = wp.tile([128, 128], dt.bfloat16)
            nc.sync.dma_start(e4_sb[:], e4[:])
            e4t_sb = wp.tile([128, 128], dt.bfloat16)
            nc.sync.dma_start(e4t_sb[:], e4t[:])
            v0_sb = wp.tile([128, 4, BC], dt.bfloat16)
            nc.sync.dma_start(v0_sb[:], v0[:])
            g0_sb = wp.tile([128, 4, BC], dt.bfloat16)
            nc.sync.dma_start(g0_sb[:], g0[:])
            b1_sb = wp.tile([128, 4], dt.bfloat16)
            nc.sync.dma_start(b1_sb[:], b1[:])
            b1t_sb = wp.tile([4, 128], dt.bfloat16)
            nc.sync.dma_start(b1t_sb[:], b1t[:])
            id_sb = wp.tile([128, 128], dt.bfloat16)
            nc.sync.dma_start(id_sb[:], ident[:])
            selz_sb = big.tile([128, BC, W2, 4], dt.bfloat16)
            nc.sync.dma_start(selz_sb[:], selz[:])
            em_sb = big.tile([128, 4, WB], dt.bfloat16)
            nc.sync.dma_start(em_sb[:], emmask[:])
            mch_sb = wp.tile([4, BC, 4], dt.float32)
            nc.sync.dma_start(mch_sb[:], mchain[:])
            mld_sb = wp.tile([4, BC, NSL, 4], dt.float32)
            nc.sync.dma_start(mld_sb[:], mled[:])
            o4_sb = wp.tile([4, 1], dt.float32)
            nc.sync.dma_start(o4_sb[:], ones4[:])
            o128_sb = wp.tile([128, 1], dt.float32)
            nc.sync.dma_start(o128_sb[:], ones128[:])

            # ---- logits: per-(pb,cg) PSUM slab -> scalar-engine copy to SBUF
            logits = big.tile([128, 4, WB], dt.float32)
            nc.vector.memset(logits[:], 0.0)
            for pb in range(4):
                for cg in range(4):
                    s = pb * 4 + cg
                    off = WST[s]
                    lgc = pslg.tile([128, 512], dt.float32, name="lgc")
                    outv = lgc[32 * pb:32 * pb + 20, 0:WB]
                    for kc in range(4):
                        rhs = hc_sb[:, kc, :].rearrange(
                            "p (t b) -> p t b", b=BC)[:, off:off + W2, :]
                        nc.tensor.matmul(
                            outv, lw_sb[:, kc, :], rhs,
                            start=(kc == 0), stop=(kc == 3),
                            tile_position=(0, 32 * pb),
                            skip_group_check=True)
                    nc.scalar.copy(
                        logits[32 * pb:32 * pb + 20, cg, :], outv)

            # exp(logits + bias) -> elog [128, 4, W*BC]
            elog = big.tile([128, 4, WB], dt.float32)
            nc.scalar.activation(elog[:], logits[:], AF.Exp, bias=lb_sb[:])

            # emission: sum over (logits .* emmask); dump product into hc_sb
            erow = sm.tile([128, 1], dt.float32)
            dump = hc_sb[:, 0, 0:4 * WB].rearrange("p (c w) -> p c w", c=4)
            nc.vector.scalar_tensor_tensor(
                dump, logits[:], 1.0, em_sb[:], AO.mult, AO.mult,
                accum_out=erow[:])
            etot = aux2[0:1, 0:1]
            nc.tensor.matmul(etot, o128_sb[:], erow[:], start=True, stop=True,
                             skip_group_check=True)
            eout = sm.tile([1, 1], dt.float32)
            nc.vector.tensor_copy(eout[:], etot)
            nc.sync.dma_start(emit_out[:], eout[:])

            elogv = elog[:].rearrange("p c (w b) -> p c w b", b=BC)

            # ---- forward + left window scans, interleaved emission so the
            # two independent chains overlap on the in-order engine FIFOs ----
            vhist = big.tile([128, W2, 4, BC], dt.bfloat16)
            shist = sm.tile([4, NSL, 4, BC], dt.float32)
            gcur = g0_sb          # SBUF bf16 gamma state (or PSUM view)
            gcur_psum = None
            gbuf = sm.tile([128, 4, BC], dt.bfloat16, name="gbuf")

            def left_phase(p, ip):
                nonlocal gcur, gcur_psum
                u = sm.tile([128, 4, BC], dt.bfloat16, name=f"gu{ip % 2}")
                src = gcur_psum if gcur_psum is not None else gcur[:]
                nc.vector.tensor_tensor(u[:], src, elogv[:, :, p, :], AO.mult)
                gy = psg.tile([128, 4 * BC], dt.float32, name="gy")
                nc.tensor.matmul(gy[:], e4t_sb[:],
                                 u[:].rearrange("p c b -> p (c b)"),
                                 start=True, stop=True)
                gcur_psum = gy[:].rearrange("p (c b) -> p c b", b=BC)
                if p % 8 == 0:
                    gsb = sm.tile([128, 4, BC], dt.bfloat16,
                                  name=f"gc{ip % 2}")
                    nc.vector.tensor_copy(gsb[:], gcur_psum)
                    ss2 = aux[0:4, :]
                    nc.tensor.matmul(ss2, b1_sb[:],
                                     gsb[:].rearrange("p c b -> p (c b)"),
                                     start=True, stop=True,
                                     skip_group_check=True)
                    rinv2 = sm.tile([4, 4 * BC], dt.bfloat16, name="ri2")
                    with nc.allow_low_precision(
                            reason="gamma direction-only renorm"):
                        nc.vector.reciprocal(rinv2[:], ss2)
                    rb2 = aux2[:]
                    nc.tensor.matmul(rb2, b1t_sb[:], rinv2[:],
                                     start=True, stop=True,
                                     skip_group_check=True)
                    gn = sm.tile([128, 4, BC], dt.bfloat16, name=f"gn{ip % 2}")
                    nc.vector.tensor_tensor(
                        gn[:], gsb[:], rb2.rearrange(
                            "p (c b) -> p c b", b=BC), AO.mult)
                    gcur, gcur_psum = gn, None

            for p in range(W2):
                y = psv.tile([128, 4 * BC], dt.float32, name="y")
                rhs = v0_sb[:].rearrange("p c b -> p (c b)") if p == 0 else \
                    vhist[:, p - 1].rearrange("p c b -> p (c b)")
                nc.tensor.matmul(y[:], e4_sb[:], rhs, start=True, stop=True)
                yv = y[:].rearrange("p (c b) -> p c b", b=BC)
                nc.vector.tensor_tensor(vhist[:, p], yv, elogv[:, :, p, :],
                                        AO.mult)
                if (p + 1) % 8 == 0:
                    r = (p + 1) // 8 - 1
                    ss = aux[0:4, :]
                    nc.tensor.matmul(
                        ss, b1_sb[:],
                        vhist[:, p].rearrange("p c b -> p (c b)"),
                        start=True, stop=True, skip_group_check=True)
                    rinv = sm.tile([4, 4 * BC], dt.bfloat16, name="rinv")
                    with nc.allow_low_precision(
                            reason="bf16 renorm scale; its exact log is "
                                   "ledgered via shist"):
                        nc.vector.reciprocal(rinv[:], ss)
                    nc.vector.tensor_copy(
                        shist[:, r], rinv[:].rearrange("p (c b) -> p c b", b=BC))
                    rb = aux2[:]
                    nc.tensor.matmul(rb, b1t_sb[:], rinv[:],
                                     start=True, stop=True,
                                     skip_group_check=True)
                    nc.vector.tensor_tensor(
                        vhist[:, p], vhist[:, p],
                        rb.rearrange("p (c b) -> p c b", b=BC), AO.mult)
                if p <= LOV:
                    left_phase(LOV - p, p)
            nc.vector.tensor_copy(gbuf[:], gcur_psum if gcur_psum is not None
                                  else gcur[:])

            # ---- handoff gather H ----
            H = sm.tile([128, 4, BC], dt.bfloat16, name="H")
            # same-pb sources at phase 31: target cg 2/3 <- cg 0/1
            nc.vector.tensor_copy(H[:, 2:4, :], vhist[:, 31, 0:2, :])
            # pb=0 targets cg 2/3 use phases 11/27 (sources s=0,1)
            nc.vector.tensor_copy(H[0:20, 2, :], vhist[0:20, 11, 0, :])
            nc.vector.tensor_copy(H[0:20, 3, :], vhist[0:20, 27, 1, :])
            # cross-pb: target (pb>=1, cg 0/1) <- (pb-1, cg+2) @31, shift +32
            # 1.0 (not 0) so untouched blocks give r=1 -> ln r = 0 (not -inf)
            HP = aux[:, 0:2 * BC].rearrange("p (c b) -> p c b", b=BC)
            nc.vector.memset(HP, 1.0)
            for pb in range(1, 4):
                for cg in range(2):
                    lhs = id_sb[32 * (pb - 1):32 * (pb - 1) + 20,
                                32 * (pb - 1):32 * (pb - 1) + 20]
                    src = vhist[32 * (pb - 1):32 * (pb - 1) + 20, 31,
                                cg + 2, :]
                    nc.tensor.matmul(
                        HP[32 * pb:32 * pb + 20, cg, :], lhs, src,
                        start=True, stop=True,
                        tile_position=(32 * (pb - 1), 32 * pb),
                        skip_group_check=True)
            nc.vector.tensor_copy(H[:, 0:2, :], HP)

            # ---- log r_s = ln(gamma.H / gamma.1) ----
            nprod = sm.tile([128, 4, BC], dt.bfloat16, name="nprod")
            nc.vector.tensor_tensor(nprod[:], gbuf[:], H[:], AO.mult)
            nm = aux[0:4, :]
            nc.tensor.matmul(nm, b1_sb[:],
                             nprod[:].rearrange("p c b -> p (c b)"),
                             start=True, stop=True, skip_group_check=True)
            dn = aux2[0:4, :]
            nc.tensor.matmul(dn, b1_sb[:],
                             gbuf[:].rearrange("p c b -> p (c b)"),
                             start=True, stop=True, skip_group_check=True)
            rin = sm.tile([4, 4 * BC], dt.float32)
            nc.vector.reciprocal(rin[:], dn)
            rr = sm.tile([4, 4, BC], dt.float32)
            nc.vector.tensor_tensor(rr[:],
                                    nm.rearrange("p (c b) -> p c b", b=BC),
                                    rin[:].rearrange("p (c b) -> p c b", b=BC),
                                    AO.mult)
            logr = sm.tile([4, 4, BC], dt.float32)
            nc.scalar.activation(logr[:], rr[:], AF.Ln)

            # ---- ledgers + lambda ----
            lnsh = sm.tile([4, NSL, 4, BC], dt.float32)
            nc.scalar.activation(lnsh[:], shist[:], AF.Ln)
            q1 = sm.tile([4, BC, 4], dt.float32)
            nc.vector.tensor_tensor(q1[:], logr[:].rearrange("p c b -> p b c"),
                                    mch_sb[:], AO.mult)
            q1r = sm.tile([4, BC], dt.float32)
            nc.vector.reduce_sum(q1r[:], q1[:], axis=mybir.AxisListType.X)
            q2 = sm.tile([4, BC, NSL, 4], dt.float32)
            nc.vector.tensor_tensor(
                q2[:], lnsh[:].rearrange("p r c b -> p b r c"), mld_sb[:],
                AO.mult)
            q2r = sm.tile([4, BC], dt.float32)
            nc.vector.reduce_sum(q2r[:], q2[:], axis=mybir.AxisListType.XY)
            lam = aux[0:1, 0:BC]
            nc.tensor.matmul(lam, o4_sb[:], q1r[:], start=True, stop=False,
                             skip_group_check=True)
            nc.tensor.matmul(lam, o4_sb[:], q2r[:], start=False, stop=True,
                             skip_group_check=True)

            # ---- extraction ----
            zp = big.tile([128, BC, W2 * 4], dt.float32)
            nc.vector.tensor_tensor(
                zp[:].rearrange("p b (w c) -> p b w c", c=4),
                vhist[:].rearrange("p w c b -> p b w c"), selz_sb[:], AO.mult)
            zr = sm.tile([128, BC], dt.float32)
            nc.vector.reduce_sum(zr[:], zp[:].rearrange(
                "p b (w c) -> p b w c", c=4), axis=mybir.AxisListType.XY)
            zs = aux2[0:1, 0:BC]
            nc.tensor.matmul(zs, o128_sb[:], zr[:], start=True, stop=True,
                             skip_group_check=True)
            lnz = sm.tile([1, BC], dt.float32)
            nc.scalar.activation(lnz[:], zs, AF.Ln)
            lamc = sm.tile([1, BC], dt.float32)
            nc.vector.tensor_copy(lamc[:], lam)
            pout = sm.tile([1, BC], dt.float32)
            nc.vector.tensor_tensor(pout[:], lnz[:], lamc[:], AO.add)
            nc.sync.dma_start(part_out[:], pout[:])
    nc.compile()
    return nc


def build_crf_program():
    bass, tile, bacc, mybir, _ = _mods()
    dt = mybir.dt
    AF = mybir.ActivationFunctionType
    AO = mybir.AluOpType

    nc = bacc.Bacc("TRN2", target_bir_lowering=False, debug=False)
    hcat = nc.dram_tensor("hcat", [4, 128, NTOK3], dt.bfloat16, kind="ExternalInput").ap()
    linw = nc.dram_tensor("linw", [4, 128, NT], dt.bfloat16, kind="ExternalInput").ap()
    linb = nc.dram_tensor("linb", [NT, 1], dt.float32, kind="ExternalInput").ap()
    etrans = nc.dram_tensor("etrans", [NT, NT], dt.float32, kind="ExternalInput").ap()
    estart = nc.dram_tensor("estart", [NT, 1], dt.float32, kind="ExternalInput").ap()
    eend = nc.dram_tensor("eend", [NT, 1], dt.float32, kind="ExternalInput").ap()
    emitmask = nc.dram_tensor("emitmask", [NT, NTOK3], dt.bfloat16, kind="ExternalInput").ap()
    lastsel = nc.dram_tensor("lastsel", [NT, BC, T], dt.bfloat16, kind="ExternalInput").ap()
    smask = nc.dram_tensor("smask", [1, BC, NREN + 1], dt.float32, kind="ExternalInput").ap()
    part_out = nc.dram_tensor("part_out", [1, BC], dt.float32, kind="ExternalOutput").ap()
    emit_out = nc.dram_tensor("emit_out", [1, 1], dt.float32, kind="ExternalOutput").ap()

    NCHUNKS3 = NTOK3 // NCH  # 4

    with tile.TileContext(nc) as tc:
        with (
            tc.tile_pool(name="w", bufs=1) as wpool,
            tc.tile_pool(name="big", bufs=1) as big,
            tc.tile_pool(name="sm", bufs=4) as sm,
            tc.tile_pool(name="pslg", bufs=2, space="PSUM") as pslg,
            tc.tile_pool(name="ps", bufs=2, space="PSUM") as ps,
        ):
            hc_sb = big.tile([128, 4, NTOK3], dt.bfloat16)
            for kc in range(4):
                nc.sync.dma_start(hc_sb[:, kc, :], hcat[kc])
            lw_sb = wpool.tile([128, 4, NT], dt.bfloat16)
            for kc in range(4):
                nc.sync.dma_start(lw_sb[:, kc, :], linw[kc])
            lb_sb = wpool.tile([NT, 1], dt.float32)
            nc.sync.dma_start(lb_sb[:], linb[:])
            et_sb = wpool.tile([NT, NT], dt.float32)
            nc.sync.dma_start(et_sb[:], etrans[:])
            es_sb = wpool.tile([NT, 1], dt.float32)
            nc.sync.dma_start(es_sb[:], estart[:])
            ee_sb = wpool.tile([NT, 1], dt.float32)
            nc.sync.dma_start(ee_sb[:], eend[:])
            em_sb = big.tile([NT, NTOK3], dt.bfloat16)
            nc.sync.dma_start(em_sb[:], emitmask[:])
            ls_sb = big.tile([NT, BC, T], dt.bfloat16)
            nc.sync.dma_start(ls_sb[:], lastsel[:])
            sm_sb = wpool.tile([1, BC, NREN + 1], dt.float32)
            nc.sync.dma_start(sm_sb[:], smask[:])
            ones_sb = wpool.tile([NT, 1], dt.float32)
            nc.vector.memset(ones_sb[:], 1.0)
            onesrow = wpool.tile([1, NT], dt.float32)
            nc.vector.memset(onesrow[:], 1.0)

            # logits^T [NT, t, b] fp32, and exp(logits)
            logits = big.tile([NT, T, BC], dt.float32)
            for n in range(NCHUNKS3):
                acc = pslg.tile([NT, NCH], dt.float32, name="lg")
                for kc in range(4):
                    nc.tensor.matmul(acc[:], lw_sb[:, kc, :],
                                     hc_sb[:, kc, n * NCH:(n + 1) * NCH],
                                     start=(kc == 0), stop=(kc == 3))
                accv = acc[:].rearrange("p (t b) -> p t b", b=BC)
                nc.vector.tensor_scalar_add(
                    logits[:, n * (NCH // BC):(n + 1) * (NCH // BC), :],
                    accv, lb_sb[:])
            elog = big.tile([NT, T, BC], dt.float32)
            nc.scalar.activation(elog[:], logits[:], AF.Exp)

            # exp-domain forward recursion, two chains of 4 sequences
            NBH = BC // 2
            shist = big.tile([1, BC, NREN + 1], dt.float32)
            nc.vector.memset(shist[:], 1.0)
            ahists = []
            for c in range(2):
                ah = big.tile([NT, NBH, T], dt.float32, name=f"ah{c}")
                nc.vector.tensor_scalar_mul(
                    ah[:, :, 0], elog[:, 0, c * NBH:(c + 1) * NBH], es_sb[:])
                ahists.append(ah)
            for t in range(1, T):
                for c in range(2):
                    ah = ahists[c]
                    bsl = slice(c * NBH, (c + 1) * NBH)
                    y = ps.tile([NT, NBH], dt.float32, name=f"y{c}", bufs=1)
                    nc.tensor.matmul(y[:], et_sb[:], ah[:, :, t - 1],
                                     start=True, stop=True)
                    if t % RENORM_EVERY == 0:
                        r = t // RENORM_EVERY - 1
                        ssum = ps.tile([NT, NBH], dt.float32, name=f"aux{c}", bufs=1)[0:1]
                        nc.tensor.matmul(ssum[:], ones_sb[:], ah[:, :, t - 1],
                                         start=True, stop=True)
                        nc.vector.tensor_copy(shist[:, bsl, r], ssum[:])
                        rinv = sm.tile([1, NBH], dt.float32, name=f"rinv{c}")
                        nc.vector.reciprocal(rinv[:], ssum[:])
                        rb = ps.tile([NT, NBH], dt.float32, name=f"aux{c}", bufs=1)
                        nc.tensor.matmul(rb[:], onesrow[:], rinv[:],
                                         start=True, stop=True)
                        u1 = sm.tile([NT, NBH], dt.float32, name=f"u1{c}")
                        nc.vector.tensor_tensor(u1[:], y[:], elog[:, t, bsl],
                                                AO.mult)
                        nc.vector.tensor_tensor(ah[:, :, t], u1[:], rb[:],
                                                AO.mult)
                    else:
                        nc.vector.tensor_tensor(ah[:, :, t], y[:],
                                                elog[:, t, bsl], AO.mult)

            # partition_b = ln(sum_j a[len_b-1, j] * e_end[j]) + sum_r ln(s_rb)
            alast = sm.tile([NT, BC], dt.float32)
            for c in range(2):
                bsl = slice(c * NBH, (c + 1) * NBH)
                prod = big.tile([NT, NBH, T], dt.float32, name=f"prod{c}")
                nc.vector.tensor_tensor(prod[:], ahists[c][:], ls_sb[:, bsl, :],
                                        AO.mult)
                nc.vector.reduce_sum(alast[:, bsl], prod[:],
                                     axis=mybir.AxisListType.X)
            w2 = sm.tile([NT, BC], dt.float32)
            nc.vector.tensor_scalar_mul(w2[:], alast[:], ee_sb[:])
            fsum = ps.tile([1, BC], dt.float32, name="faux", bufs=1)
            nc.tensor.matmul(fsum[:], ones_sb[:], w2[:], start=True, stop=True)
            pln = sm.tile([1, BC], dt.float32)
            nc.scalar.activation(pln[:], fsum[:], AF.Ln)
            slog = sm.tile([1, BC, NREN + 1], dt.float32)
            nc.scalar.activation(slog[:], shist[:], AF.Ln)
            slogm = sm.tile([1, BC, NREN + 1], dt.float32)
            nc.vector.tensor_tensor(slogm[:], slog[:], sm_sb[:], AO.mult)
            zb = sm.tile([1, BC], dt.float32)
            nc.vector.reduce_sum(zb[:], slogm[:], axis=mybir.AxisListType.X)
            pout = sm.tile([1, BC], dt.float32)
            nc.vector.tensor_tensor(pout[:], pln[:], zb[:], AO.add)
            nc.sync.dma_start(part_out[:], pout[:])

            # emission score total
            eprod = big.tile([NT, T, BC], dt.float32)
            nc.vector.tensor_tensor(
                eprod[:], logits[:],
                em_sb[:].rearrange("p (t b) -> p t b", b=BC), AO.mult)
            erow = sm.tile([NT, 1], dt.float32)
            nc.vector.reduce_sum(erow[:], eprod[:], axis=mybir.AxisListType.XY)
            etot = ps.tile([1, 1], dt.float32, name="faux", bufs=1)
            nc.tensor.matmul(etot[:], ones_sb[:], erow[:], start=True, stop=True)
            eout = sm.tile([1, 1], dt.float32)
            nc.vector.tensor_copy(eout[:], etot[:])
            nc.sync.dma_start(emit_out[:], eout[:])
    nc.compile()
    return nc


# --------------------------------------------------------------------------
# host-side data prep
# --------------------------------------------------------------------------

def _crf_v2_consts(trans, start_t, end_t, lin_b):
    """Input tensors shared by all cores for the v2 CRF launch."""
    E = np.exp(trans.astype(np.float64))
    e4 = np.zeros((128, 128), np.float32)
    e4t = np.zeros((128, 128), np.float32)
    for pb in range(4):
        e4[32 * pb:32 * pb + 20, 32 * pb:32 * pb + 20] = E
        e4t[32 * pb:32 * pb + 20, 32 * pb:32 * pb + 20] = E.T
    alpha_m1 = np.linalg.solve(E.T, np.exp(start_t.astype(np.float64)))
    v0 = np.zeros((128, 4, BC), np.float32)
    g0 = np.zeros((128, 4, BC), np.float32)
    for pb in range(4):
        for cg in range(4):
            s = pb * 4 + cg
            init = alpha_m1 if s < 2 else np.ones(NT)
            v0[32 * pb:32 * pb + 20, cg, :] = init[:, None]
            g0[32 * pb:32 * pb + 20, cg, :] = 1.0
    b1 = np.zeros((128, 4), np.float32)
    b1t = np.zeros((4, 128), np.float32)
    for pb in range(4):
        b1[32 * pb:32 * pb + 20, pb] = 1.0
        b1t[pb, 32 * pb:32 * pb + 20] = 1.0
    identb = np.eye(128, dtype=np.float32)
    linb4 = np.zeros((128, 1), np.float32)
    for pb in range(4):
        linb4[32 * pb:32 * pb + 20, 0] = lin_b
    return {
        "e4": e4.astype(BF16), "e4t": e4t.astype(BF16),
        "v0": v0.astype(BF16), "g0": g0.astype(BF16),
        "b1": b1.astype(BF16), "b1t": b1t.astype(BF16),
        "identb": identb.astype(BF16), "linb4": linb4,
        "ones4": np.ones((4, 1), np.float32),
        "ones128": np.ones((128, 1), np.float32),
    }


def _crf_v2_seq_masks(lens_c, labels_c, end_t):
    """selz / emmask / mchain / mled for one core's BC sequences."""
    ee = np.exp(end_t.astype(np.float64)).astype(np.float32)
    selz = np.zeros((128, BC, W2, 4), np.float32)
    emmask = np.zeros((128, 4, W2, BC), np.float32)
    mchain = np.zeros((4, BC, 4), np.float32)
    mled = np.zeros((4, BC, NSL, 4), np.float32)
    for b in range(BC):
        L = int(lens_c[b]) - 1
        sb = min(L // LSEG, S2 - 1)
        pph = L - WST[sb]
        pb_b, cg_b = divmod(sb, 4)
        selz[32 * pb_b:32 * pb_b + 20, b, pph, cg_b] = ee
        # extraction renorm ledger
        for r in range(NSL):
            if pph >= 8 * r + 7:
                mled[pb_b, b, r, cg_b] += 1.0
        # chain
        s = sb
        while s >= 2:
            pbs, cgs = divmod(s, 4)
            mchain[pbs, b, cgs] = 1.0
            sp, pp = SRC_SLOT[s]
            pbp, cgp = divmod(sp, 4)
            for r in range(NSL):
                if pp >= 8 * r + 7:
                    mled[pbp, b, r, cgp] += 1.0
            s = sp
        # emission ownership
        for t in range(int(lens_c[b])):
            s = t // LSEG
            pbs, cgs = divmod(s, 4)
            p = t - WST[s]
            emmask[32 * pbs + int(labels_c[b, t]), cgs, p, b] = 1.0
    # shist holds the bf16 reciprocal actually applied, so its log enters
    # the ledger with the opposite sign
    return (np.ascontiguousarray(selz).astype(BF16),
            np.ascontiguousarray(emmask.reshape(128, 4, W2 * BC)).astype(BF16),
            mchain, -mled)


def _layer_inputs(xin, w_ih, w_hh, b_ih, b_hh):
    """Per-core input dicts for one layer launch.

    xin: [2, B, T, K] fp32 (xin[1] already reversed+masked)
    w_ih: [2, 4HD, K]; w_hh: [2, 4HD, HD]; b_ih, b_hh: [2, 4HD]
    """
    K = xin.shape[-1]
    kc_in = K // 128
    # scale the g-gate rows (post-perm block 3) by 2: tanh(x) = 2*sig(2x)-1
    gscale = np.ones((4 * HD, 1), np.float32)
    gscale[2 * HD:3 * HD] = 2.0
    per_dir = []
    for d in range(2):
        wih_p = w_ih[d][_PERM] * gscale
        whh_p = w_hh[d][_PERM] * gscale
        b_p = (b_ih[d] + b_hh[d])[_PERM] * gscale[:, 0]
        wihT = np.ascontiguousarray(
            wih_p.T.reshape(kc_in, 128, 4 * HD)).astype(BF16)
        whhT = np.ascontiguousarray(
            whh_p.T.reshape(2, 128, 4 * HD)).astype(BF16)
        bs = np.ascontiguousarray(
            b_p.reshape(NJ, 128).T).astype(np.float32)
        per_dir.append((wihT, whhT, bs))
    maps = []
    for core in range(NCORES):
        d, q = divmod(core, 4)
        xc = xin[d, q * BL:(q + 1) * BL]              # [BL, T, K]
        xT = np.ascontiguousarray(
            xc.transpose(2, 1, 0).reshape(kc_in, 128, T * BL)).astype(BF16)
        wihT, whhT, bs = per_dir[d]
        maps.append({"xT": xT, "wih": wihT, "whh": whhT, "bias": bs})
    return maps


def _collect_h(results):
    """per-core 'hout' [128,2,T,BL] bf16 -> h [2, B, T, HD] fp32."""
    h = np.empty((2, B, T, HD), np.float32)
    for core in range(NCORES):
        d, q = divmod(core, 4)
        ho = np.asarray(results[core]["hout"], dtype=np.float32)
        h[d, q * BL:(q + 1) * BL] = ho.transpose(3, 2, 1, 0).reshape(BL, T, HD)
    return h


def _unreverse(h_rev, lens, valid):
    """h_rev[b, s] holds position lens_b-1-s; return h[b, t] (zeros at pad)."""
    t = np.arange(T)
    idx = np.clip(lens[:, None] - 1 - t[None, :], 0, T - 1)
    out = np.take_along_axis(h_rev, idx[:, :, None], axis=1)
    return out * valid[:, :, None]


def kernel(**inputs):
    _, _, _, _, run_bass_kernel_spmd = _mods()
    global LAST_RESULTS
    LAST_RESULTS = []
    trace = bool(int(os.environ.get("KERNEL_TRACE", "0")))
    if trace:
        _install_ntff_shim()

    tokens = np.asarray(inputs["tokens"]).astype(np.int64)
    lens = np.asarray(inputs["lens"]).astype(np.int64)
    labels = np.asarray(inputs["labels"]).astype(np.int64)
    emb = np.asarray(inputs["emb"], dtype=np.float32)
    w_ih = [np.asarray(inputs["w_ih_l0"], np.float32),
            np.asarray(inputs["w_ih_l1"], np.float32)]
    w_hh = [np.asarray(inputs["w_hh_l0"], np.float32),
            np.asarray(inputs["w_hh_l1"], np.float32)]
    b_ih = [np.asarray(inputs["b_ih_l0"], np.float32),
            np.asarray(inputs["b_ih_l1"], np.float32)]
    b_hh = [np.asarray(inputs["b_hh_l0"], np.float32),
            np.asarray(inputs["b_hh_l1"], np.float32)]
    lin_w = np.asarray(inputs["lin_w"], np.float32)
    lin_b = np.asarray(inputs["lin_b"], np.float32)
    trans = np.asarray(inputs["trans"], np.float32)
    start_t = np.asarray(inputs["start_t"], np.float32)
    end_t = np.asarray(inputs["end_t"], np.float32)

    t_ar = np.arange(T)
    valid = (t_ar[None, :] < lens[:, None]).astype(np.float32)
    rev_idx = np.clip(lens[:, None] - 1 - t_ar[None, :], 0, T - 1)

    if "layer0" not in _CACHE:
        _CACHE["layer0"] = build_layer_program(E // 128)
    if "layer1" not in _CACHE:
        _CACHE["layer1"] = build_layer_program(2 * HD // 128)
    if "crf2" not in _CACHE:
        _CACHE["crf2"] = build_crf_program_v2()

    cores = list(range(NCORES))

    # ---------- launch 1: layer 0 ----------
    x = emb[tokens]
    x_rev = np.take_along_axis(x, rev_idx[:, :, None], axis=1) * valid[:, :, None]
    xin0 = np.stack([x, x_rev])
    res1 = run_bass_kernel_spmd(
        _CACHE["layer0"], _layer_inputs(xin0, w_ih[0], w_hh[0], b_ih[0], b_hh[0]),
        cores, trace=trace)
    LAST_RESULTS.append(res1)
    h0 = _collect_h(res1.results)

    # ---------- launch 2: layer 1 ----------
    h0f = h0[0] * valid[:, :, None]
    h0b = _unreverse(h0[1], lens, valid)
    x1 = np.concatenate([h0f, h0b], axis=-1)
    x1_rev = np.take_along_axis(x1, rev_idx[:, :, None], axis=1) * valid[:, :, None]
    xin1 = np.stack([x1, x1_rev])
    res2 = run_bass_kernel_spmd(
        _CACHE["layer1"], _layer_inputs(xin1, w_ih[1], w_hh[1], b_ih[1], b_hh[1]),
        cores, trace=trace)
    LAST_RESULTS.append(res2)
    h1 = _collect_h(res2.results)

    # ---------- launch 3: logits + CRF ----------
    h1f = h1[0] * valid[:, :, None]
    h1b = _unreverse(h1[1], lens, valid)
    hcat = np.concatenate([h1f, h1b], axis=-1)

    lw = np.ascontiguousarray(lin_w.T.reshape(4, 128, NT)).astype(BF16)
    consts = _crf_v2_consts(trans, start_t, end_t, lin_b)
    maps = []
    for core in range(NCORES):
        bs = slice(core * BC, (core + 1) * BC)
        hc = hcat[bs]
        hcT = np.ascontiguousarray(
            hc.transpose(2, 1, 0).reshape(4, 128, T * BC)).astype(BF16)
        selz, emmask, mchain, mled = _crf_v2_seq_masks(
            lens[bs], labels[bs], end_t)
        m = {"hcat": hcT, "linw": lw, "selz": selz, "emmask": emmask,
             "mchain": mchain, "mled": mled}
        m.update(consts)
        maps.append(m)
    res3 = run_bass_kernel_spmd(_CACHE["crf2"], maps, cores, trace=trace)
    LAST_RESULTS.append(res3)

    partition = np.concatenate(
        [np.asarray(r["part_out"])[0] for r in res3.results])
    emit = float(sum(np.asarray(r["emit_out"])[0, 0] for r in res3.results))

    # host-side numerator terms (incl. the logits-bias part of the emission
    # score: the device emission uses bias-free logits)
    first_tag = labels[:, 0]
    last_tag = np.take_along_axis(labels, (lens - 1)[:, None], axis=1)[:, 0]
    tr_sc = float((trans[labels[:, :-1], labels[:, 1:]] * valid[:, 1:]).sum())
    emit_b = float((lin_b[labels] * valid).sum())
    host_num = (float(start_t[first_tag].sum()) + tr_sc
                + float(end_t[last_tag].sum()) + emit_b)

    loss = partition.sum() - emit - host_num
    return np.float32(loss)

